# revision 7
# baseline (speedup 1.0000x reference)
"""Trainium2 Bass kernel for nn_BLP_52467320487972 (retrieval_knn, L1 scores).

score[b, e] = -sum_d |query_sum[b, d] - E_embed[e, d]|,
E_embed = [other_emb[0]; ent_pkl @ proj_W.T]

Strategy (8 NeuronCores, entity-sharded, 5000(+pad) entities/core):
  host:   exact query_sum [32, 256] (tiny gather + normalize); score column 0;
          per-core ent shard transposed to [768, 5120] bf16.
  device: bf16 projection on the PE produces P.T half-tiles [128d, E];
          the 32 query rows are split across engines to balance load:
            - DVE bf16 lane: relu(P - q) via 4x tensor_scalar; PE column-sums
              each bf16 half-tile with a 2.0-staircase (|x| = 2 relu(x) - x);
            - DVE fp8 lane: same relu emitted fp8e4 (2x mode); ONE fp8
              DoubleRow matmul per row reduces both halves (pair dim) at
              0.5 cycles/column - 4x cheaper than the bf16 reduction;
            - ACT lane: |P - q| directly via activation(Abs, bias=-q) in fp8,
              DoubleRow-reduced (no relu identity, so no colsum correction);
            - Pool (GPSIMD) lane: relu tensor_scalar in fp8, DoubleRow reduce.
          A negones matmul per half adds the "- sum x" correction only to
          relu-lane rows (host later adds their sum(q) term).
          Projection / PSUM-copy run at 512-entity sub-group granularity and
          group g+1's projection is emitted before group g's row work so the
          PE and the PSUM->SBUF copies never serialize the pipeline.
  host:   stitch score columns, negate, prepend column 0.
"""

import sys

for _p in ("/opt/trn_rl_repo", "/root/.axon_site/_ro/trn_rl_repo"):
    if _p not in sys.path:
        sys.path.append(_p)

import numpy as np
import ml_dtypes

NUM_ENT = 40000
NUM_REL = 100
EMBED_DIM = 256
FEAT_DIM = 768
BATCH = 32
N_CORES = 8
SHARD = NUM_ENT // N_CORES          # 5000
SHARD_PAD = 5120                    # 40 tiles of 128
K_CHUNKS = FEAT_DIM // 128          # 6
GROUP = 1024
N_GROUPS = SHARD_PAD // GROUP       # 5
SUB = 512                           # proj/copy/matmul chunk
EPS = 1e-12

# Query-row assignment (relu rows first so the negones columns are a range):
N_DVEB = 9                          # DVE bf16 relu rows (bf16 stair reduce)
N_DVE8 = 10                         # DVE fp8 relu rows (DoubleRow reduce)
N_POOL = 6                          # Pool fp8 relu rows (DoubleRow reduce)
N_ACT = BATCH - N_DVEB - N_DVE8 - N_POOL  # 7 ACT abs rows (DoubleRow reduce)
N_RELU = N_DVEB + N_DVE8 + N_POOL

ROWS_DVEB = list(range(0, N_DVEB))
ROWS_DVE8 = list(range(N_DVEB, N_DVEB + N_DVE8))
ROWS_POOL = list(range(N_DVEB + N_DVE8, N_RELU))
ROWS_ACT = list(range(N_RELU, BATCH))

BF16 = ml_dtypes.bfloat16
FP8 = ml_dtypes.float8_e4m3

_CACHE = {}


def _build_program():
    import concourse.bacc as bacc
    import concourse.mybir as mybir
    import concourse.tile as tile

    f32 = mybir.dt.float32
    bf16 = mybir.dt.bfloat16
    fp8 = mybir.dt.float8e4
    AL = mybir.AluOpType
    ACT = mybir.ActivationFunctionType
    DR = mybir.MatmulPerfMode.DoubleRow

    nc = bacc.Bacc("TRN2", target_bir_lowering=False, debug=False, num_devices=N_CORES)
    a_t = nc.declare_dram_parameter("a_t", [128, K_CHUNKS, SHARD_PAD], bf16, isOutput=False)
    w_t = nc.declare_dram_parameter("w_t", [128, 2, K_CHUNKS, 128], bf16, isOutput=False)
    qt = nc.declare_dram_parameter("qt", [128, 2, BATCH], f32, isOutput=False)
    qtn = nc.declare_dram_parameter("qtn", [128, 2, BATCH], f32, isOutput=False)
    stair8 = nc.declare_dram_parameter(
        "stair8", [128, BATCH, 2, BATCH], fp8, isOutput=False)
    st_out = nc.declare_dram_parameter("st_out", [BATCH, SHARD_PAD], f32, isOutput=True)

    with tile.TileContext(nc) as tc:
        with (
            tc.tile_pool(name="const", bufs=1) as const_pool,
            tc.tile_pool(name="pt", bufs=3) as pt_pool,
            tc.tile_pool(name="tb", bufs=6) as tb_pool,
            tc.tile_pool(name="t8", bufs=10) as t8_pool,
            tc.tile_pool(name="sr", bufs=2) as sr_pool,
            tc.tile_pool(name="psumt", bufs=2, space="PSUM") as psumt_pool,
            tc.tile_pool(name="psums", bufs=2, space="PSUM") as psums_pool,
        ):
            # ---- resident constants ----
            w_sb = const_pool.tile([128, 2, K_CHUNKS, 128], bf16)
            nc.sync.dma_start(out=w_sb[:], in_=w_t[:])
            qt_sb = const_pool.tile([128, 2, BATCH], f32)
            nc.sync.dma_start(out=qt_sb[:], in_=qt[:])
            qtn_sb = const_pool.tile([128, 2, BATCH], f32)
            nc.sync.dma_start(out=qtn_sb[:], in_=qtn[:])
            stair8_sb = const_pool.tile([128, BATCH, 2, BATCH], fp8)
            nc.sync.dma_start(out=stair8_sb[:], in_=stair8[:])
            stairb = const_pool.tile([128, 2 * BATCH - 1], bf16)
            nc.gpsimd.memset(stairb[:], 0.0)
            nc.gpsimd.memset(stairb[:, BATCH - 1 : BATCH], 2.0)
            negones = const_pool.tile([128, BATCH], bf16)
            nc.gpsimd.memset(negones[:], 0.0)
            nc.gpsimd.memset(negones[:, :N_RELU], -1.0)

            a_all = const_pool.tile([128, K_CHUNKS, SHARD_PAD], bf16)
            for g in range(N_GROUPS):
                for s in range(GROUP // SUB):
                    o = GROUP * g + SUB * s
                    nc.sync.dma_start(
                        out=a_all[:, :, o : o + SUB],
                        in_=a_t[:, :, o : o + SUB],
                    )

            gsizes = [GROUP] * (N_GROUPS - 1) + [SHARD - GROUP * (N_GROUPS - 1)]
            pt_tiles = [None] * N_GROUPS

            def emit_proj(g):
                """Projection + PSUM->SBUF copy for group g, SUB granularity."""
                g0 = g * GROUP
                gsz = gsizes[g]
                ssz = [SUB] * (gsz // SUB) + ([gsz % SUB] if gsz % SUB else [])
                pt_sb = pt_pool.tile([128, 2, GROUP], bf16, tag="pt")
                pt_tiles[g] = pt_sb
                for s in range(len(ssz)):
                    o = SUB * s
                    ptp = psumt_pool.tile([128, 2, SUB], f32, tag="ptp")
                    for h in range(2):
                        for k in range(K_CHUNKS):
                            nc.tensor.matmul(
                                ptp[:, h, : ssz[s]],
                                w_sb[:, h, k, :],
                                a_all[:, k, g0 + o : g0 + o + ssz[s]],
                                start=(k == 0),
                                stop=(k == K_CHUNKS - 1),
                            )
                    nc.scalar.copy(
                        pt_sb[:, :, o : o + ssz[s]], ptp[:, :, : ssz[s]])

            def emit_rows(g):
                g0 = g * GROUP
                gsz = gsizes[g]
                csz = [SUB] * (gsz // SUB) + ([gsz % SUB] if gsz % SUB else [])
                coff = [sum(csz[:i]) for i in range(len(csz))]
                nch = len(csz)
                pt_sb = pt_tiles[g]

                psum_s = psums_pool.tile([BATCH, GROUP], f32, tag="psum_s")
                started = [False] * nch

                def red_dr(b, tile8):
                    for c in range(nch):
                        nc.tensor.matmul(
                            psum_s[:, coff[c] : coff[c] + csz[c]],
                            stair8_sb[:, b],
                            tile8[:, :, coff[c] : coff[c] + csz[c]],
                            start=not started[c],
                            stop=False,
                            perf_mode=DR,
                            skip_group_check=True,
                        )
                        started[c] = True

                def red_bf(b, tileb, h):
                    for c in range(nch):
                        nc.tensor.matmul(
                            psum_s[:, coff[c] : coff[c] + csz[c]],
                            stairb[:, BATCH - 1 - b : 2 * BATCH - 1 - b],
                            tileb[:, h, coff[c] : coff[c] + csz[c]],
                            start=not started[c],
                            stop=False,
                            skip_group_check=True,
                        )
                        started[c] = True

                order = []
                mx = max(N_DVEB, N_DVE8, N_POOL, N_ACT)
                for i in range(mx):
                    for rows, kind in (
                        (ROWS_ACT, "act"),
                        (ROWS_DVEB, "dveb"),
                        (ROWS_POOL, "pool"),
                        (ROWS_DVE8, "dve8"),
                    ):
                        if i < len(rows):
                            order.append((rows[i], kind))

                for b, kind in order:
                    if kind == "dveb":
                        tl = tb_pool.tile([128, 2, GROUP], bf16, tag="tb")
                        for h in range(2):
                            nc.vector.tensor_scalar(
                                out=tl[:, h, :gsz],
                                in0=pt_sb[:, h, :gsz],
                                scalar1=qt_sb[:, h, b : b + 1],
                                scalar2=0.0,
                                op0=AL.subtract,
                                op1=AL.max,
                            )
                            red_bf(b, tl, h)
                    elif kind == "dve8":
                        tl = t8_pool.tile([128, 2, GROUP], fp8, tag="t8d")
                        for h in range(2):
                            nc.vector.tensor_scalar(
                                out=tl[:, h, :gsz],
                                in0=pt_sb[:, h, :gsz],
                                scalar1=qt_sb[:, h, b : b + 1],
                                scalar2=0.0,
                                op0=AL.subtract,
                                op1=AL.max,
                            )
                        red_dr(b, tl)
                    elif kind == "pool":
                        tl = t8_pool.tile([128, 2, GROUP], fp8, tag="t8p")
                        for h in range(2):
                            nc.gpsimd.tensor_scalar(
                                out=tl[:, h, :gsz],
                                in0=pt_sb[:, h, :gsz],
                                scalar1=qt_sb[:, h, b : b + 1],
                                scalar2=0.0,
                                op0=AL.subtract,
                                op1=AL.max,
                            )
                        red_dr(b, tl)
                    else:  # act
                        tl = t8_pool.tile([128, 2, GROUP], fp8, tag="t8a")
                        for h in range(2):
                            nc.scalar.activation(
                                tl[:, h, :gsz],
                                pt_sb[:, h, :gsz],
                                ACT.Abs,
                                bias=qtn_sb[:, h, b : b + 1],
                                scale=1.0,
                            )
                        red_dr(b, tl)

                # "- sum x" correction for relu rows (negones columns)
                for h in range(2):
                    for c in range(nch):
                        nc.tensor.matmul(
                            psum_s[:, coff[c] : coff[c] + csz[c]],
                            negones[:],
                            pt_sb[:, h, coff[c] : coff[c] + csz[c]],
                            start=False,
                            stop=(h == 1),
                            skip_group_check=True,
                        )

                sr = sr_pool.tile([BATCH, GROUP], f32, tag="sr")
                nc.scalar.copy(sr[:, :gsz], psum_s[:, :gsz])
                nc.sync.dma_start(out=st_out[:, g0 : g0 + gsz], in_=sr[:, :gsz])

            # software pipelining: group g+1's projection is emitted (and thus
            # queued on PE/ACT) ahead of group g's row work.
            emit_proj(0)
            for g in range(N_GROUPS):
                if g + 1 < N_GROUPS:
                    emit_proj(g + 1)
                emit_rows(g)

    nc.compile()
    return nc


def _get_program():
    if "nc" not in _CACHE:
        _CACHE["nc"] = _build_program()
    return _CACHE["nc"]


def _host_query_sum(ent_pkl, other_emb, proj_W, batch_input_ids, mp):
    """Exact replica of the reference's query path, on host (64 rows only)."""
    ids = np.concatenate([batch_input_ids[:, :mp], batch_input_ids[:, mp + 1 : 3]], axis=1)
    ids = ids.astype(np.int64)  # [B, 2]
    q = np.empty((BATCH, 2, EMBED_DIM), dtype=np.float32)
    for b in range(BATCH):
        for j in range(2):
            idx = int(ids[b, j])
            if idx == 0:
                row = other_emb[0]
            elif idx <= NUM_ENT:
                row = ent_pkl[idx - 1].astype(np.float32) @ proj_W.T.astype(np.float32)
            else:
                row = other_emb[idx - NUM_ENT]
            q[b, j] = row
    norm = np.sqrt((q * q).sum(-1, keepdims=True))
    q = q / np.maximum(norm, EPS)
    return q.sum(axis=1)  # [B, 256] float32


def kernel(ent_pkl, other_emb, proj_W, batch_input_ids, batch_mask_position, _timing=None):
    from concourse.bass_utils import run_bass_kernel_spmd

    ent_pkl = np.asarray(ent_pkl, dtype=np.float32)
    other_emb = np.asarray(other_emb, dtype=np.float32)
    proj_W = np.asarray(proj_W, dtype=np.float32)
    batch_input_ids = np.asarray(batch_input_ids)
    mp = int(np.asarray(batch_mask_position))

    q_sum = _host_query_sum(ent_pkl, other_emb, proj_W, batch_input_ids, mp)

    # score column 0: entity row = other_emb[0]
    col0 = -np.abs(q_sum - other_emb[0][None, :]).sum(-1)  # [B]

    # ---- device input prep ----
    # w_t[kp, h, k, m] = proj_W.T[128k+kp, 128h+m]
    w_full = np.ascontiguousarray(proj_W.T)  # [768, 256]
    w_np = np.ascontiguousarray(
        w_full.reshape(K_CHUNKS, 128, 2, 128).transpose(1, 2, 0, 3)).astype(BF16)

    # qt[kp, h, b] = q_sum[b, 128h+kp]
    qth = np.transpose(q_sum.T.reshape(2, 128, BATCH), (1, 0, 2))  # [128, 2, 32]
    qt_np = np.ascontiguousarray(qth.astype(np.float32))
    qtn_np = np.ascontiguousarray((-qth).astype(np.float32))

    # per-row DoubleRow stair: 2.0 for relu-identity rows, 1.0 for abs rows
    stair8_np = np.zeros((128, BATCH, 2, BATCH), dtype=FP8)
    for b in range(BATCH):
        stair8_np[:, b, :, b] = FP8(2.0) if b < N_RELU else FP8(1.0)

    a_t_full = ent_pkl.T.astype(BF16)  # [768, 40000]
    in_maps = []
    for c in range(N_CORES):
        shard_t = a_t_full[:, c * SHARD : (c + 1) * SHARD]  # [768, 5000]
        a_np = np.zeros((128, K_CHUNKS, SHARD_PAD), dtype=BF16)
        a_np[:, :, :SHARD] = shard_t.reshape(K_CHUNKS, 128, SHARD).transpose(1, 0, 2)
        in_maps.append({
            "a_t": a_np,
            "w_t": w_np,
            "qt": qt_np,
            "qtn": qtn_np,
            "stair8": stair8_np,
        })

    nc = _get_program()
    kwargs = dict(_timing) if _timing else {}
    res = run_bass_kernel_spmd(nc, in_maps, list(range(N_CORES)), **kwargs)
    if _timing is not None:
        _CACHE["last_results"] = res

    qsum = q_sum.sum(-1).astype(np.float32)  # [B]
    s_ent = np.empty((BATCH, NUM_ENT), dtype=np.float32)
    for c in range(N_CORES):
        sl = slice(c * SHARD, (c + 1) * SHARD)
        s_ent[:, sl] = res.results[c]["st_out"][:, :SHARD]
    # relu-identity rows still need the +sum(q) term
    s_ent[:N_RELU] += qsum[:N_RELU, None]
    out = np.empty((BATCH, NUM_ENT + 1), dtype=np.float32)
    out[:, 0] = col0
    out[:, 1:] = -s_ent
    return out


# revision 23
# speedup vs baseline: 1.1090x; 1.1090x over previous
"""Trainium2 Bass kernel for nn_BLP_52467320487972 (retrieval_knn, L1 scores).

score[b, e] = -sum_d |query_sum[b, d] - E_embed[e, d]|,
E_embed = [other_emb[0]; ent_pkl @ proj_W.T]

Strategy (8 NeuronCores, entity-sharded, 5000(+pad) entities/core):
  host:   exact query_sum [32, 256] (tiny gather + normalize); score column 0;
          per-core ent shard transposed to [768, 5120] bf16.
  device: bf16 projection on the PE produces P.T half-tiles [128d, E];
          the 32 query rows are split across engines to balance load:
            - DVE bf16 lane: relu(P - q) via 4x tensor_scalar; PE column-sums
              each bf16 half-tile with a 2.0-staircase (|x| = 2 relu(x) - x);
            - DVE fp8 lane: same relu emitted fp8e4 (2x mode); ONE fp8
              DoubleRow matmul per row reduces both halves (pair dim) at
              0.5 cycles/column - 4x cheaper than the bf16 reduction;
            - ACT lane: |P - q| directly via activation(Abs, bias=-q) in fp8,
              DoubleRow-reduced (no relu identity, so no colsum correction);
            - Pool (GPSIMD) lane: relu tensor_scalar in fp8, DoubleRow reduce.
          A negones matmul per half adds the "- sum x" correction only to
          relu-lane rows (host later adds their sum(q) term).
          Projection / PSUM-copy run at 512-entity sub-group granularity and
          group g+1's projection is emitted before group g's row work so the
          PE and the PSUM->SBUF copies never serialize the pipeline.
  host:   stitch score columns, negate, prepend column 0.
"""

import sys

for _p in ("/opt/trn_rl_repo", "/root/.axon_site/_ro/trn_rl_repo"):
    if _p not in sys.path:
        sys.path.append(_p)

import numpy as np
import ml_dtypes

NUM_ENT = 40000
NUM_REL = 100
EMBED_DIM = 256
FEAT_DIM = 768
BATCH = 32
N_CORES = 8
SHARD = NUM_ENT // N_CORES          # 5000
SHARD_PAD = 5120                    # 40 tiles of 128
K_CHUNKS = FEAT_DIM // 128          # 6
GROUP = 1024
N_GROUPS = SHARD_PAD // GROUP       # 5
SUB = 512                           # proj/copy/matmul chunk
EPS = 1e-12

# Query-row assignment (relu rows first so the negones columns are a range):
N_DVEB = 8                          # DVE bf16 relu rows (bf16 stair reduce)
N_DVE8 = 11                         # DVE fp8 relu rows (DoubleRow reduce)
N_POOL = 6                          # Pool fp8 relu rows (DoubleRow reduce)
N_ACT = BATCH - N_DVEB - N_DVE8 - N_POOL  # 7 ACT abs rows (DoubleRow reduce)
N_RELU = N_DVEB + N_DVE8 + N_POOL

ROWS_DVEB = list(range(0, N_DVEB))
ROWS_DVE8 = list(range(N_DVEB, N_DVEB + N_DVE8))
ROWS_POOL = list(range(N_DVEB + N_DVE8, N_RELU))
ROWS_ACT = list(range(N_RELU, BATCH))

BF16 = ml_dtypes.bfloat16
FP8 = ml_dtypes.float8_e4m3

_CACHE = {}


# Tunables (overridable for perf sweeps via _build_program(cfg=...)):
DEFAULT_CFG = {
    "split": (N_DVEB, N_DVE8, N_POOL),  # (dveb, dve8, pool); act = rest
    "emit": "tiles_first",              # or "interleave"
    "pt_bufs": 3,
    "sub": SUB,
    "psums_bufs": 4,
    "first_dma_split": 128,
    "tail_pool_move": 0,
    "tail_pool_lane": "dveb",
    "tail_sr_dve": True,
    "gsched": (256, 768, 1024, 1024, 1024, 904),
    "filler_at": (None, None),
}


def _build_program(cfg=None):
    import concourse.bacc as bacc
    import concourse.mybir as mybir
    import concourse.tile as tile

    cfg = {**DEFAULT_CFG, **(cfg or {})}
    n_dveb, n_dve8, n_pool = cfg["split"]
    n_relu = n_dveb + n_dve8 + n_pool
    n_act = BATCH - n_relu
    rows_dveb = list(range(0, n_dveb))
    rows_dve8 = list(range(n_dveb, n_dveb + n_dve8))
    rows_pool = list(range(n_dveb + n_dve8, n_relu))
    rows_act = list(range(n_relu, BATCH))
    sub = cfg["sub"]

    f32 = mybir.dt.float32
    bf16 = mybir.dt.bfloat16
    fp8 = mybir.dt.float8e4
    AL = mybir.AluOpType
    ACT = mybir.ActivationFunctionType
    DR = mybir.MatmulPerfMode.DoubleRow

    nc = bacc.Bacc("TRN2", target_bir_lowering=False, debug=False, num_devices=N_CORES)
    a_t = nc.declare_dram_parameter("a_t", [128, K_CHUNKS, SHARD_PAD], bf16, isOutput=False)
    w_t = nc.declare_dram_parameter("w_t", [128, 2, K_CHUNKS, 128], bf16, isOutput=False)
    qt = nc.declare_dram_parameter("qt", [128, 2, BATCH], f32, isOutput=False)
    qtn = nc.declare_dram_parameter("qtn", [128, 2, BATCH], f32, isOutput=False)
    stair8 = nc.declare_dram_parameter(
        "stair8", [128, BATCH, 2, BATCH], fp8, isOutput=False)
    st_out = nc.declare_dram_parameter("st_out", [BATCH, SHARD_PAD], f32, isOutput=True)

    # per-lane tile pools so buffer counts track each lane's live-tile needs
    tf = cfg["emit"] == "tiles_first"
    tb_bufs = n_dveb + 2 if tf else 6
    t8d_bufs = n_dve8 + 2 if tf else 6
    t8p_bufs = n_pool + 2 if tf else 4
    t8a_bufs = n_act + 2 if tf else 4

    with tile.TileContext(nc) as tc:
        with (
            tc.tile_pool(name="const", bufs=1) as const_pool,
            tc.tile_pool(name="pt", bufs=cfg["pt_bufs"]) as pt_pool,
            tc.tile_pool(name="tb", bufs=tb_bufs) as tb_pool,
            tc.tile_pool(name="t8d", bufs=t8d_bufs) as t8d_pool,
            tc.tile_pool(name="t8p", bufs=t8p_bufs) as t8p_pool,
            tc.tile_pool(name="t8a", bufs=t8a_bufs) as t8a_pool,
            tc.tile_pool(name="sr", bufs=4) as sr_pool,
            tc.tile_pool(name="psumt", bufs=2, space="PSUM") as psumt_pool,
            tc.tile_pool(name="psums", bufs=cfg["psums_bufs"], space="PSUM") as psums_pool,
        ):
            # ---- resident constants ----
            # DMA order matters: w + the first A chunk go first on the sync
            # queue (they gate the first matmul); qt/qtn/stair8 ride the ACT
            # engine's DGE queue in parallel.
            a_all = const_pool.tile([128, K_CHUNKS, SHARD_PAD], bf16)
            fds = cfg["first_dma_split"]
            dma_offs = []
            if fds and fds < sub:
                dma_offs += [(0, fds), (fds, sub - fds)]
            else:
                dma_offs += [(0, sub)]
            for i in range(1, N_GROUPS * (GROUP // sub)):
                dma_offs.append((i * sub, sub))
            nc.sync.dma_start(
                out=a_all[:, :, : dma_offs[0][1]], in_=a_t[:, :, : dma_offs[0][1]])
            w_sb = const_pool.tile([128, 2, K_CHUNKS, 128], bf16)
            nc.scalar.dma_start(out=w_sb[:, 0], in_=w_t[:, 0])
            nc.scalar.dma_start(out=w_sb[:, 1], in_=w_t[:, 1])

            qt_sb = const_pool.tile([128, 2, BATCH], f32)
            nc.gpsimd.dma_start(out=qt_sb[:], in_=qt[:])
            qtn_sb = const_pool.tile([128, 2, BATCH], f32)
            nc.gpsimd.dma_start(out=qtn_sb[:], in_=qtn[:])
            stair8_sb = const_pool.tile([128, BATCH, 2, BATCH], fp8)
            nc.scalar.dma_start(out=stair8_sb[:], in_=stair8[:])
            stairb = const_pool.tile([128, 2 * BATCH - 1], bf16)
            nc.gpsimd.memset(stairb[:], 0.0)
            nc.gpsimd.memset(stairb[:, BATCH - 1 : BATCH], 2.0)
            negones = const_pool.tile([128, BATCH], bf16)
            nc.gpsimd.memset(negones[:], 0.0)
            nc.gpsimd.memset(negones[:, :n_relu], -1.0)

            for o, s in dma_offs[1:]:
                nc.sync.dma_start(
                    out=a_all[:, :, o : o + s], in_=a_t[:, :, o : o + s])

            gsizes = list(cfg["gsched"])
            assert sum(gsizes) == SHARD
            n_groups = len(gsizes)
            goffs = [sum(gsizes[:i]) for i in range(n_groups)]
            pt_tiles = [None] * n_groups

            def proj_pieces(g):
                """Projection + PSUM->SBUF copy for group g as per-sub
                emission pieces (interleavable into the PE stream)."""
                g0 = goffs[g]
                gsz = gsizes[g]
                ssz = [sub] * (gsz // sub) + ([gsz % sub] if gsz % sub else [])
                if g == 0 and fds and fds < ssz[0]:
                    ssz = [fds, ssz[0] - fds] + ssz[1:]
                pt_sb = pt_pool.tile([128, 2, GROUP], bf16, tag="pt")
                pt_tiles[g] = pt_sb

                soff = [sum(ssz[:i]) for i in range(len(ssz))]

                def piece(s):
                    o = soff[s]
                    ptp = psumt_pool.tile([128, 2, sub], f32, tag="ptp")
                    for h in range(2):
                        for k in range(K_CHUNKS):
                            nc.tensor.matmul(
                                ptp[:, h, : ssz[s]],
                                w_sb[:, h, k, :],
                                a_all[:, k, g0 + o : g0 + o + ssz[s]],
                                start=(k == 0),
                                stop=(k == K_CHUNKS - 1),
                            )
                    nc.scalar.copy(
                        pt_sb[:, :, o : o + ssz[s]], ptp[:, :, : ssz[s]])

                return [lambda s=s: piece(s) for s in range(len(ssz))]

            def emit_rows(g, filler):
                g0 = goffs[g]
                gsz = gsizes[g]
                csz = [sub] * (gsz // sub) + ([gsz % sub] if gsz % sub else [])
                coff = [sum(csz[:i]) for i in range(len(csz))]
                nch = len(csz)
                pt_sb = pt_tiles[g]
                last = g == n_groups - 1

                # per-group lane map: the tail group sheds Pool rows (slowest
                # producer) onto the DVE fp8 lane so the drain isn't
                # Pool-paced.
                kinds = {}
                for b in rows_dveb:
                    kinds[b] = "dveb"
                for b in rows_dve8:
                    kinds[b] = "dve8"
                for i, b in enumerate(rows_pool):
                    kinds[b] = (cfg["tail_pool_lane"]
                                if last and i < cfg["tail_pool_move"]
                                else "pool")
                for b in rows_act:
                    kinds[b] = "act"

                def make_tile(b):
                    kind = kinds[b]
                    if kind == "dveb":
                        tl = tb_pool.tile([128, 2, GROUP], bf16, tag="tb")
                        eng = nc.vector
                    elif kind == "dve8":
                        tl = t8d_pool.tile([128, 2, GROUP], fp8, tag="t8d")
                        eng = nc.vector
                    elif kind == "pool":
                        tl = t8p_pool.tile([128, 2, GROUP], fp8, tag="t8p")
                        eng = nc.gpsimd
                    else:
                        tl = t8a_pool.tile([128, 2, GROUP], fp8, tag="t8a")
                        eng = None
                    for h in range(2):
                        if eng is None:
                            nc.scalar.activation(
                                tl[:, h, :gsz], pt_sb[:, h, :gsz], ACT.Abs,
                                bias=qtn_sb[:, h, b : b + 1], scale=1.0)
                        else:
                            eng.tensor_scalar(
                                out=tl[:, h, :gsz], in0=pt_sb[:, h, :gsz],
                                scalar1=qt_sb[:, h, b : b + 1], scalar2=0.0,
                                op0=AL.subtract, op1=AL.max)
                    return tl

                # production emission: round-robin across lanes
                prod_order = []
                by_kind = {"dveb": [], "dve8": [], "pool": [], "act": []}
                for b in range(BATCH):
                    by_kind[kinds[b]].append(b)
                mx = max(len(v) for v in by_kind.values())
                for i in range(mx):
                    for kind in ("act", "pool", "dveb", "dve8"):
                        if i < len(by_kind[kind]):
                            prod_order.append(by_kind[kind][i])

                tiles = {}
                for b in prod_order:
                    tiles[b] = make_tile(b)

                # reduction order: estimated tile completion per lane stream
                per_row = {"dveb": 0.66, "dve8": 1.19, "act": 2.08, "pool": 3.04}
                clock = {k: 0.0 for k in per_row}
                done_at = {}
                for b in prod_order:
                    clock[kinds[b]] += per_row[kinds[b]]
                    done_at[b] = clock[kinds[b]]
                red_order = sorted(done_at, key=done_at.get)

                for c in range(nch):
                    lo, sz = coff[c], csz[c]
                    psum_c = psums_pool.tile([BATCH, sub], f32, tag="psum_c")
                    first = True
                    fpos = cfg["filler_at"]
                    filler_at = (fpos[c] if c < len(fpos) else fpos[-1])
                    if filler_at is None:
                        filler_at = max(
                            (i for i, b in enumerate(red_order)
                             if kinds[b] in ("dveb", "dve8")), default=0)
                    for i, b in enumerate(red_order):
                        kind = kinds[b]
                        tl = tiles[b]
                        if kind == "dveb":
                            for h in range(2):
                                nc.tensor.matmul(
                                    psum_c[:, :sz],
                                    stairb[:, BATCH - 1 - b : 2 * BATCH - 1 - b],
                                    tl[:, h, lo : lo + sz],
                                    start=first, stop=False,
                                    skip_group_check=True)
                                first = False
                        else:
                            nc.tensor.matmul(
                                psum_c[:, :sz],
                                stair8_sb[:, b],
                                tl[:, :, lo : lo + sz],
                                start=first, stop=False,
                                perf_mode=DR, skip_group_check=True)
                            first = False
                        if i == filler_at and filler:
                            filler.pop(0)()
                    for h in range(2):
                        nc.tensor.matmul(
                            psum_c[:, :sz],
                            negones[:],
                            pt_sb[:, h, lo : lo + sz],
                            start=False, stop=(h == 1),
                            skip_group_check=True)
                    sr = sr_pool.tile([BATCH, sub], f32, tag="sr")
                    if last and cfg["tail_sr_dve"]:
                        nc.vector.tensor_copy(out=sr[:, :sz], in_=psum_c[:, :sz])
                    else:
                        nc.scalar.copy(sr[:, :sz], psum_c[:, :sz])
                    nc.sync.dma_start(
                        out=st_out[:, g0 + lo : g0 + lo + sz], in_=sr[:, :sz])
                while filler:
                    filler.pop(0)()

            # software pipelining: group g+1's projection pieces are slotted
            # into group g's reduction stream as PE filler work.
            for p in proj_pieces(0):
                p()
            for g in range(n_groups):
                filler = proj_pieces(g + 1) if g + 1 < n_groups else []
                emit_rows(g, filler)

    nc.compile()
    return nc


def _get_program():
    if "nc" not in _CACHE:
        _CACHE["nc"] = _build_program()
    return _CACHE["nc"]


def _host_query_sum(ent_pkl, other_emb, proj_W, batch_input_ids, mp):
    """Exact replica of the reference's query path, on host (64 rows only)."""
    ids = np.concatenate([batch_input_ids[:, :mp], batch_input_ids[:, mp + 1 : 3]], axis=1)
    ids = ids.astype(np.int64)  # [B, 2]
    q = np.empty((BATCH, 2, EMBED_DIM), dtype=np.float32)
    for b in range(BATCH):
        for j in range(2):
            idx = int(ids[b, j])
            if idx == 0:
                row = other_emb[0]
            elif idx <= NUM_ENT:
                row = ent_pkl[idx - 1].astype(np.float32) @ proj_W.T.astype(np.float32)
            else:
                row = other_emb[idx - NUM_ENT]
            q[b, j] = row
    norm = np.sqrt((q * q).sum(-1, keepdims=True))
    q = q / np.maximum(norm, EPS)
    return q.sum(axis=1)  # [B, 256] float32


def kernel(ent_pkl, other_emb, proj_W, batch_input_ids, batch_mask_position, _timing=None):
    from concourse.bass_utils import run_bass_kernel_spmd

    ent_pkl = np.asarray(ent_pkl, dtype=np.float32)
    other_emb = np.asarray(other_emb, dtype=np.float32)
    proj_W = np.asarray(proj_W, dtype=np.float32)
    batch_input_ids = np.asarray(batch_input_ids)
    mp = int(np.asarray(batch_mask_position))

    q_sum = _host_query_sum(ent_pkl, other_emb, proj_W, batch_input_ids, mp)

    # score column 0: entity row = other_emb[0]
    col0 = -np.abs(q_sum - other_emb[0][None, :]).sum(-1)  # [B]

    # ---- device input prep ----
    # w_t[kp, h, k, m] = proj_W.T[128k+kp, 128h+m]
    w_full = np.ascontiguousarray(proj_W.T)  # [768, 256]
    w_np = np.ascontiguousarray(
        w_full.reshape(K_CHUNKS, 128, 2, 128).transpose(1, 2, 0, 3)).astype(BF16)

    # qt[kp, h, b] = q_sum[b, 128h+kp]
    qth = np.transpose(q_sum.T.reshape(2, 128, BATCH), (1, 0, 2))  # [128, 2, 32]
    qt_np = np.ascontiguousarray(qth.astype(np.float32))
    qtn_np = np.ascontiguousarray((-qth).astype(np.float32))

    # per-row DoubleRow stair: 2.0 for relu-identity rows, 1.0 for abs rows
    stair8_np = np.zeros((128, BATCH, 2, BATCH), dtype=FP8)
    for b in range(BATCH):
        stair8_np[:, b, :, b] = FP8(2.0) if b < N_RELU else FP8(1.0)

    a_t_full = ent_pkl.T.astype(BF16)  # [768, 40000]
    in_maps = []
    for c in range(N_CORES):
        shard_t = a_t_full[:, c * SHARD : (c + 1) * SHARD]  # [768, 5000]
        a_np = np.zeros((128, K_CHUNKS, SHARD_PAD), dtype=BF16)
        a_np[:, :, :SHARD] = shard_t.reshape(K_CHUNKS, 128, SHARD).transpose(1, 0, 2)
        in_maps.append({
            "a_t": a_np,
            "w_t": w_np,
            "qt": qt_np,
            "qtn": qtn_np,
            "stair8": stair8_np,
        })

    nc = _get_program()
    kwargs = dict(_timing) if _timing else {}
    res = run_bass_kernel_spmd(nc, in_maps, list(range(N_CORES)), **kwargs)
    if _timing is not None:
        _CACHE["last_results"] = res

    qsum = q_sum.sum(-1).astype(np.float32)  # [B]
    s_ent = np.empty((BATCH, NUM_ENT), dtype=np.float32)
    for c in range(N_CORES):
        sl = slice(c * SHARD, (c + 1) * SHARD)
        s_ent[:, sl] = res.results[c]["st_out"][:, :SHARD]
    # relu-identity rows still need the +sum(q) term
    s_ent[:N_RELU] += qsum[:N_RELU, None]
    out = np.empty((BATCH, NUM_ENT + 1), dtype=np.float32)
    out[:, 0] = col0
    out[:, 1:] = -s_ent
    return out


# revision 33
# speedup vs baseline: 1.1201x; 1.0100x over previous
"""Trainium2 Bass kernel for nn_BLP_52467320487972 (retrieval_knn, L1 scores).

score[b, e] = -sum_d |query_sum[b, d] - E_embed[e, d]|,
E_embed = [other_emb[0]; ent_pkl @ proj_W.T]

Strategy (8 NeuronCores, entity-sharded, 5000(+pad) entities/core):
  host:   exact query_sum [32, 256] (tiny gather + normalize); score column 0;
          per-core ent shard transposed to [768, 5120] bf16.
  device: bf16 projection on the PE produces P.T half-tiles [128d, E];
          the 32 query rows are split across engines to balance load:
            - DVE bf16 lane: relu(P - q) via 4x tensor_scalar; PE column-sums
              each bf16 half-tile with a 2.0-staircase (|x| = 2 relu(x) - x);
            - DVE fp8 lane: same relu emitted fp8e4 (2x mode); ONE fp8
              DoubleRow matmul per row reduces both halves (pair dim) at
              0.5 cycles/column - 4x cheaper than the bf16 reduction;
            - ACT lane: |P - q| directly via activation(Abs, bias=-q) in fp8,
              DoubleRow-reduced (no relu identity, so no colsum correction);
            - Pool (GPSIMD) lane: relu tensor_scalar in fp8, DoubleRow reduce.
          A negones matmul per half adds the "- sum x" correction only to
          relu-lane rows (host later adds their sum(q) term).
          Projection / PSUM-copy run at 512-entity sub-group granularity and
          group g+1's projection is emitted before group g's row work so the
          PE and the PSUM->SBUF copies never serialize the pipeline.
  host:   stitch score columns, negate, prepend column 0.
"""

import sys

for _p in ("/opt/trn_rl_repo", "/root/.axon_site/_ro/trn_rl_repo"):
    if _p not in sys.path:
        sys.path.append(_p)

import numpy as np
import ml_dtypes

NUM_ENT = 40000
NUM_REL = 100
EMBED_DIM = 256
FEAT_DIM = 768
BATCH = 32
N_CORES = 8
SHARD = NUM_ENT // N_CORES          # 5000
SHARD_PAD = 5120                    # 40 tiles of 128
K_CHUNKS = FEAT_DIM // 128          # 6
GROUP = 1024
N_GROUPS = SHARD_PAD // GROUP       # 5
SUB = 512                           # proj/copy/matmul chunk
EPS = 1e-12

# Query-row assignment (relu rows first so the negones columns are a range):
N_DVEB = 8                          # DVE bf16 relu rows (bf16 stair reduce)
N_DVE8 = 11                         # DVE fp8 relu rows (DoubleRow reduce)
N_POOL = 6                          # Pool fp8 relu rows (DoubleRow reduce)
N_ACT = BATCH - N_DVEB - N_DVE8 - N_POOL  # 7 ACT abs rows (DoubleRow reduce)
N_RELU = N_DVEB + N_DVE8 + N_POOL

ROWS_DVEB = list(range(0, N_DVEB))
ROWS_DVE8 = list(range(N_DVEB, N_DVEB + N_DVE8))
ROWS_POOL = list(range(N_DVEB + N_DVE8, N_RELU))
ROWS_ACT = list(range(N_RELU, BATCH))

BF16 = ml_dtypes.bfloat16
FP8 = ml_dtypes.float8_e4m3

_CACHE = {}


# Tunables (overridable for perf sweeps via _build_program(cfg=...)):
DEFAULT_CFG = {
    "split": (N_DVEB, N_DVE8, N_POOL),  # (dveb, dve8, pool); act = rest
    "emit": "tiles_first",              # or "interleave"
    "pt_bufs": 3,
    "sub": SUB,
    "psums_bufs": 4,
    "first_dma_split": 128,
    "tail_pool_move": 2,
    "tail_pool_lane": "actr",
    "tail_sr_dve": True,
    "gsched": (256, 768, 1024, 1024, 1024, 904),
    "filler_at": (None, None),
    "warmup": 40,
    "lane_sched": None,
    "bufscale": 1.0,
}


def _build_program(cfg=None):
    import concourse.bacc as bacc
    import concourse.mybir as mybir
    import concourse.tile as tile

    cfg = {**DEFAULT_CFG, **(cfg or {})}
    n_dveb, n_dve8, n_pool = cfg["split"]
    n_relu = n_dveb + n_dve8 + n_pool
    n_act = BATCH - n_relu
    rows_dveb = list(range(0, n_dveb))
    rows_dve8 = list(range(n_dveb, n_dveb + n_dve8))
    rows_pool = list(range(n_dveb + n_dve8, n_relu))
    rows_act = list(range(n_relu, BATCH))
    sub = cfg["sub"]

    f32 = mybir.dt.float32
    bf16 = mybir.dt.bfloat16
    fp8 = mybir.dt.float8e4
    AL = mybir.AluOpType
    ACT = mybir.ActivationFunctionType
    DR = mybir.MatmulPerfMode.DoubleRow

    nc = bacc.Bacc("TRN2", target_bir_lowering=False, debug=False, num_devices=N_CORES)
    a_t = nc.declare_dram_parameter("a_t", [128, K_CHUNKS, SHARD_PAD], bf16, isOutput=False)
    w_t = nc.declare_dram_parameter("w_t", [128, 2, K_CHUNKS, 128], bf16, isOutput=False)
    qt = nc.declare_dram_parameter("qt", [128, 2, BATCH], f32, isOutput=False)
    qtn = nc.declare_dram_parameter("qtn", [128, 2, BATCH], f32, isOutput=False)
    stair8 = nc.declare_dram_parameter(
        "stair8", [128, BATCH, 2, BATCH], fp8, isOutput=False)
    st_out = nc.declare_dram_parameter("st_out", [BATCH, SHARD_PAD], f32, isOutput=True)

    # per-lane tile pools so buffer counts track each lane's live-tile needs
    tf = cfg["emit"] == "tiles_first"
    bs = cfg["bufscale"]
    tb_bufs = max(2, int((n_dveb + 2) * bs)) if tf else 6
    t8d_bufs = max(2, int((n_dve8 + 2) * bs)) if tf else 6
    t8p_bufs = max(2, int((n_pool + 2) * bs)) if tf else 4
    t8a_bufs = max(2, int((n_act + 2) * bs)) if tf else 4

    with tile.TileContext(nc) as tc:
        with (
            tc.tile_pool(name="const", bufs=1) as const_pool,
            tc.tile_pool(name="pt", bufs=cfg["pt_bufs"]) as pt_pool,
            tc.tile_pool(name="tb", bufs=tb_bufs) as tb_pool,
            tc.tile_pool(name="t8d", bufs=t8d_bufs) as t8d_pool,
            tc.tile_pool(name="t8p", bufs=t8p_bufs) as t8p_pool,
            tc.tile_pool(name="t8a", bufs=t8a_bufs) as t8a_pool,
            tc.tile_pool(name="sr", bufs=4) as sr_pool,
            tc.tile_pool(name="psumt", bufs=2, space="PSUM") as psumt_pool,
            tc.tile_pool(name="psums", bufs=cfg["psums_bufs"], space="PSUM") as psums_pool,
        ):
            # ---- resident constants ----
            # DMA order matters: w + the first A chunk go first on the sync
            # queue (they gate the first matmul); qt/qtn/stair8 ride the ACT
            # engine's DGE queue in parallel.
            a_all = const_pool.tile([128, K_CHUNKS, SHARD_PAD], bf16)
            fds = cfg["first_dma_split"]
            dma_offs = []
            if fds and fds < sub:
                dma_offs += [(0, fds), (fds, sub - fds)]
            else:
                dma_offs += [(0, sub)]
            for i in range(1, N_GROUPS * (GROUP // sub)):
                dma_offs.append((i * sub, sub))
            nc.sync.dma_start(
                out=a_all[:, :, : dma_offs[0][1]], in_=a_t[:, :, : dma_offs[0][1]])
            w_sb = const_pool.tile([128, 2, K_CHUNKS, 128], bf16)
            nc.scalar.dma_start(out=w_sb[:, 0], in_=w_t[:, 0])
            nc.scalar.dma_start(out=w_sb[:, 1], in_=w_t[:, 1])

            qt_sb = const_pool.tile([128, 2, BATCH], f32)
            nc.gpsimd.dma_start(out=qt_sb[:], in_=qt[:])
            qtn_sb = const_pool.tile([128, 2, BATCH], f32)
            nc.gpsimd.dma_start(out=qtn_sb[:], in_=qtn[:])
            stair8_sb = const_pool.tile([128, BATCH, 2, BATCH], fp8)
            nc.scalar.dma_start(out=stair8_sb[:], in_=stair8[:])
            stairb = const_pool.tile([128, 2 * BATCH - 1], bf16)
            nc.gpsimd.memset(stairb[:], 0.0)
            nc.gpsimd.memset(stairb[:, BATCH - 1 : BATCH], 2.0)
            negones = const_pool.tile([128, BATCH], bf16)
            nc.gpsimd.memset(negones[:], 0.0)
            nc.gpsimd.memset(negones[:, :n_relu], -1.0)

            for o, s in dma_offs[1:]:
                nc.sync.dma_start(
                    out=a_all[:, :, o : o + s], in_=a_t[:, :, o : o + s])

            if cfg["warmup"]:
                # dummy matmuls ramp the PE pstate while the first A/W DMAs
                # are in flight, so the real projection starts at full clock
                wu = const_pool.tile([128, 512], bf16)
                nc.gpsimd.memset(wu[:], 0.0)
                wups = psums_pool.tile([64, 512], f32, tag="psum_c")
                for i in range(cfg["warmup"]):
                    nc.tensor.matmul(
                        wups[:, :64], wu[:, :64], wu[:, :64],
                        start=True, stop=True, skip_group_check=True)

            gsizes = list(cfg["gsched"])
            assert sum(gsizes) == SHARD
            gmax = max(max(gsizes), GROUP)
            n_groups = len(gsizes)
            goffs = [sum(gsizes[:i]) for i in range(n_groups)]
            pt_tiles = [None] * n_groups

            def proj_pieces(g):
                """Projection + PSUM->SBUF copy for group g as per-sub
                emission pieces (interleavable into the PE stream)."""
                g0 = goffs[g]
                gsz = gsizes[g]
                ssz = [sub] * (gsz // sub) + ([gsz % sub] if gsz % sub else [])
                if g == 0 and fds and fds < ssz[0]:
                    ssz = [fds, ssz[0] - fds] + ssz[1:]
                pt_sb = pt_pool.tile([128, 2, gmax], bf16, tag="pt")
                pt_tiles[g] = pt_sb

                soff = [sum(ssz[:i]) for i in range(len(ssz))]

                def piece(s):
                    o = soff[s]
                    ptp = psumt_pool.tile([128, 2, sub], f32, tag="ptp")
                    for h in range(2):
                        for k in range(K_CHUNKS):
                            nc.tensor.matmul(
                                ptp[:, h, : ssz[s]],
                                w_sb[:, h, k, :],
                                a_all[:, k, g0 + o : g0 + o + ssz[s]],
                                start=(k == 0),
                                stop=(k == K_CHUNKS - 1),
                            )
                    nc.scalar.copy(
                        pt_sb[:, :, o : o + ssz[s]], ptp[:, :, : ssz[s]])

                return [lambda s=s: piece(s) for s in range(len(ssz))]

            def emit_rows(g, filler):
                g0 = goffs[g]
                gsz = gsizes[g]
                csz = [sub] * (gsz // sub) + ([gsz % sub] if gsz % sub else [])
                coff = [sum(csz[:i]) for i in range(len(csz))]
                nch = len(csz)
                pt_sb = pt_tiles[g]
                last = g == n_groups - 1

                # per-group lane map. All relu rows (0..n_relu-1) may move
                # freely between the dveb/dve8/pool lanes per group (same
                # stair value + negones + host handling); lane_sched lets the
                # schedule be pool-heavy early and DVE-heavy late so the tail
                # drains at DVE speed.
                ls = cfg["lane_sched"]
                if ls is not None:
                    gb, g8, gp = ls[g]
                    assert gb + g8 + gp == n_relu
                else:
                    gb, g8, gp = n_dveb, n_dve8, n_pool
                    if last and cfg["tail_pool_move"]:
                        m = cfg["tail_pool_move"]
                        gp -= m
                        if cfg["tail_pool_lane"] == "dveb":
                            gb += m
                        elif cfg["tail_pool_lane"] == "dve8":
                            g8 += m
                        else:
                            gp += 0  # rows become "actr" below
                kinds = {}
                for b in range(n_relu):
                    kinds[b] = ("dveb" if b < gb
                                else "dve8" if b < gb + g8 else "pool")
                if ls is None and last and cfg["tail_pool_move"] and \
                        cfg["tail_pool_lane"] == "actr":
                    for b in range(n_relu - cfg["tail_pool_move"], n_relu):
                        kinds[b] = "actr"
                for b in rows_act:
                    kinds[b] = "act"

                def make_tile(b):
                    kind = kinds[b]
                    if kind == "dveb":
                        tl = tb_pool.tile([128, 2, gmax], bf16, tag="tb")
                        eng = nc.vector
                    elif kind == "dve8":
                        tl = t8d_pool.tile([128, 2, gmax], fp8, tag="t8d")
                        eng = nc.vector
                    elif kind == "pool":
                        tl = t8p_pool.tile([128, 2, gmax], fp8, tag="t8p")
                        eng = nc.gpsimd
                    else:
                        tl = t8a_pool.tile([128, 2, gmax], fp8, tag="t8a")
                        eng = None
                    func = ACT.Relu if kinds[b] == "actr" else ACT.Abs
                    for h in range(2):
                        if eng is None:
                            nc.scalar.activation(
                                tl[:, h, :gsz], pt_sb[:, h, :gsz], func,
                                bias=qtn_sb[:, h, b : b + 1], scale=1.0)
                        else:
                            eng.tensor_scalar(
                                out=tl[:, h, :gsz], in0=pt_sb[:, h, :gsz],
                                scalar1=qt_sb[:, h, b : b + 1], scalar2=0.0,
                                op0=AL.subtract, op1=AL.max)
                    return tl

                # production emission: round-robin across lanes
                prod_order = []
                by_kind = {"dveb": [], "dve8": [], "pool": [], "act": [],
                           "actr": []}
                for b in range(BATCH):
                    by_kind[kinds[b]].append(b)
                mx = max(len(v) for v in by_kind.values())
                for i in range(mx):
                    for kind in ("act", "actr", "pool", "dveb", "dve8"):
                        if i < len(by_kind[kind]):
                            prod_order.append(by_kind[kind][i])

                tiles = {}
                for b in prod_order:
                    tiles[b] = make_tile(b)

                # reduction order: estimated tile completion per lane stream
                per_row = {"dveb": 0.66, "dve8": 1.19, "act": 2.08,
                           "actr": 2.08, "pool": 3.04}
                clock = {k: 0.0 for k in per_row}
                done_at = {}
                for b in prod_order:
                    clock[kinds[b]] += per_row[kinds[b]]
                    done_at[b] = clock[kinds[b]]
                red_order = sorted(done_at, key=done_at.get)

                for c in range(nch):
                    lo, sz = coff[c], csz[c]
                    psum_c = psums_pool.tile([BATCH, sub], f32, tag="psum_c")
                    first = True
                    fpos = cfg["filler_at"]
                    filler_at = (fpos[c] if c < len(fpos) else fpos[-1])
                    if filler_at is None:
                        filler_at = max(
                            (i for i, b in enumerate(red_order)
                             if kinds[b] in ("dveb", "dve8")), default=0)
                    for i, b in enumerate(red_order):
                        kind = kinds[b]
                        tl = tiles[b]
                        if kind == "dveb":
                            for h in range(2):
                                nc.tensor.matmul(
                                    psum_c[:, :sz],
                                    stairb[:, BATCH - 1 - b : 2 * BATCH - 1 - b],
                                    tl[:, h, lo : lo + sz],
                                    start=first, stop=False,
                                    skip_group_check=True)
                                first = False
                        else:
                            nc.tensor.matmul(
                                psum_c[:, :sz],
                                stair8_sb[:, b],
                                tl[:, :, lo : lo + sz],
                                start=first, stop=False,
                                perf_mode=DR, skip_group_check=True)
                            first = False
                        if i == filler_at and filler:
                            filler.pop(0)()
                    for h in range(2):
                        nc.tensor.matmul(
                            psum_c[:, :sz],
                            negones[:],
                            pt_sb[:, h, lo : lo + sz],
                            start=False, stop=(h == 1),
                            skip_group_check=True)
                    sr = sr_pool.tile([BATCH, sub], f32, tag="sr")
                    if last and cfg["tail_sr_dve"]:
                        nc.vector.tensor_copy(out=sr[:, :sz], in_=psum_c[:, :sz])
                    else:
                        nc.scalar.copy(sr[:, :sz], psum_c[:, :sz])
                    nc.sync.dma_start(
                        out=st_out[:, g0 + lo : g0 + lo + sz], in_=sr[:, :sz])
                while filler:
                    filler.pop(0)()

            # software pipelining: group g+1's projection pieces are slotted
            # into group g's reduction stream as PE filler work.
            for p in proj_pieces(0):
                p()
            for g in range(n_groups):
                filler = proj_pieces(g + 1) if g + 1 < n_groups else []
                emit_rows(g, filler)

    nc.compile()
    return nc


def _get_program():
    if "nc" not in _CACHE:
        _CACHE["nc"] = _build_program()
    return _CACHE["nc"]


def _host_query_sum(ent_pkl, other_emb, proj_W, batch_input_ids, mp):
    """Exact replica of the reference's query path, on host (64 rows only)."""
    ids = np.concatenate([batch_input_ids[:, :mp], batch_input_ids[:, mp + 1 : 3]], axis=1)
    ids = ids.astype(np.int64)  # [B, 2]
    q = np.empty((BATCH, 2, EMBED_DIM), dtype=np.float32)
    for b in range(BATCH):
        for j in range(2):
            idx = int(ids[b, j])
            if idx == 0:
                row = other_emb[0]
            elif idx <= NUM_ENT:
                row = ent_pkl[idx - 1].astype(np.float32) @ proj_W.T.astype(np.float32)
            else:
                row = other_emb[idx - NUM_ENT]
            q[b, j] = row
    norm = np.sqrt((q * q).sum(-1, keepdims=True))
    q = q / np.maximum(norm, EPS)
    return q.sum(axis=1)  # [B, 256] float32


def kernel(ent_pkl, other_emb, proj_W, batch_input_ids, batch_mask_position, _timing=None):
    from concourse.bass_utils import run_bass_kernel_spmd

    ent_pkl = np.asarray(ent_pkl, dtype=np.float32)
    other_emb = np.asarray(other_emb, dtype=np.float32)
    proj_W = np.asarray(proj_W, dtype=np.float32)
    batch_input_ids = np.asarray(batch_input_ids)
    mp = int(np.asarray(batch_mask_position))

    q_sum = _host_query_sum(ent_pkl, other_emb, proj_W, batch_input_ids, mp)

    # score column 0: entity row = other_emb[0]
    col0 = -np.abs(q_sum - other_emb[0][None, :]).sum(-1)  # [B]

    # ---- device input prep ----
    # w_t[kp, h, k, m] = proj_W.T[128k+kp, 128h+m]
    w_full = np.ascontiguousarray(proj_W.T)  # [768, 256]
    w_np = np.ascontiguousarray(
        w_full.reshape(K_CHUNKS, 128, 2, 128).transpose(1, 2, 0, 3)).astype(BF16)

    # qt[kp, h, b] = q_sum[b, 128h+kp]
    qth = np.transpose(q_sum.T.reshape(2, 128, BATCH), (1, 0, 2))  # [128, 2, 32]
    qt_np = np.ascontiguousarray(qth.astype(np.float32))
    qtn_np = np.ascontiguousarray((-qth).astype(np.float32))

    # per-row DoubleRow stair: 2.0 for relu-identity rows, 1.0 for abs rows
    stair8_np = np.zeros((128, BATCH, 2, BATCH), dtype=FP8)
    for b in range(BATCH):
        stair8_np[:, b, :, b] = FP8(2.0) if b < N_RELU else FP8(1.0)

    a_t_full = ent_pkl.T.astype(BF16)  # [768, 40000]
    in_maps = []
    for c in range(N_CORES):
        shard_t = a_t_full[:, c * SHARD : (c + 1) * SHARD]  # [768, 5000]
        a_np = np.zeros((128, K_CHUNKS, SHARD_PAD), dtype=BF16)
        a_np[:, :, :SHARD] = shard_t.reshape(K_CHUNKS, 128, SHARD).transpose(1, 0, 2)
        in_maps.append({
            "a_t": a_np,
            "w_t": w_np,
            "qt": qt_np,
            "qtn": qtn_np,
            "stair8": stair8_np,
        })

    nc = _get_program()
    kwargs = dict(_timing) if _timing else {}
    res = run_bass_kernel_spmd(nc, in_maps, list(range(N_CORES)), **kwargs)
    if _timing is not None:
        _CACHE["last_results"] = res

    qsum = q_sum.sum(-1).astype(np.float32)  # [B]
    s_ent = np.empty((BATCH, NUM_ENT), dtype=np.float32)
    for c in range(N_CORES):
        sl = slice(c * SHARD, (c + 1) * SHARD)
        s_ent[:, sl] = res.results[c]["st_out"][:, :SHARD]
    # relu-identity rows still need the +sum(q) term
    s_ent[:N_RELU] += qsum[:N_RELU, None]
    out = np.empty((BATCH, NUM_ENT + 1), dtype=np.float32)
    out[:, 0] = col0
    out[:, 1:] = -s_ent
    return out


# revision 34
# speedup vs baseline: 1.1225x; 1.0022x over previous
"""Trainium2 Bass kernel for nn_BLP_52467320487972 (retrieval_knn, L1 scores).

score[b, e] = -sum_d |query_sum[b, d] - E_embed[e, d]|,
E_embed = [other_emb[0]; ent_pkl @ proj_W.T]

Strategy (8 NeuronCores, entity-sharded, 5000(+pad) entities/core):
  host:   exact query_sum [32, 256] (tiny gather + normalize); score column 0;
          per-core ent shard transposed to [768, 5120] bf16.
  device: bf16 projection on the PE produces P.T half-tiles [128d, E];
          the 32 query rows are split across engines to balance load:
            - DVE bf16 lane: relu(P - q) via 4x tensor_scalar; PE column-sums
              each bf16 half-tile with a 2.0-staircase (|x| = 2 relu(x) - x);
            - DVE fp8 lane: same relu emitted fp8e4 (2x mode); ONE fp8
              DoubleRow matmul per row reduces both halves (pair dim) at
              0.5 cycles/column - 4x cheaper than the bf16 reduction;
            - ACT lane: |P - q| directly via activation(Abs, bias=-q) in fp8,
              DoubleRow-reduced (no relu identity, so no colsum correction);
            - Pool (GPSIMD) lane: relu tensor_scalar in fp8, DoubleRow reduce.
          A negones matmul per half adds the "- sum x" correction only to
          relu-lane rows (host later adds their sum(q) term).
          Projection / PSUM-copy run at 512-entity sub-group granularity and
          group g+1's projection is emitted before group g's row work so the
          PE and the PSUM->SBUF copies never serialize the pipeline.
  host:   stitch score columns, negate, prepend column 0.
"""

import sys

for _p in ("/opt/trn_rl_repo", "/root/.axon_site/_ro/trn_rl_repo"):
    if _p not in sys.path:
        sys.path.append(_p)

import numpy as np
import ml_dtypes

NUM_ENT = 40000
NUM_REL = 100
EMBED_DIM = 256
FEAT_DIM = 768
BATCH = 32
N_CORES = 8
SHARD = NUM_ENT // N_CORES          # 5000
SHARD_PAD = 5120                    # 40 tiles of 128
K_CHUNKS = FEAT_DIM // 128          # 6
GROUP = 1024
N_GROUPS = SHARD_PAD // GROUP       # 5
SUB = 512                           # proj/copy/matmul chunk
EPS = 1e-12

# Query-row assignment (relu rows first so the negones columns are a range):
N_DVEB = 8                          # DVE bf16 relu rows (bf16 stair reduce)
N_DVE8 = 11                         # DVE fp8 relu rows (DoubleRow reduce)
N_POOL = 6                          # Pool fp8 relu rows (DoubleRow reduce)
N_ACT = BATCH - N_DVEB - N_DVE8 - N_POOL  # 7 ACT abs rows (DoubleRow reduce)
N_RELU = N_DVEB + N_DVE8 + N_POOL

ROWS_DVEB = list(range(0, N_DVEB))
ROWS_DVE8 = list(range(N_DVEB, N_DVEB + N_DVE8))
ROWS_POOL = list(range(N_DVEB + N_DVE8, N_RELU))
ROWS_ACT = list(range(N_RELU, BATCH))

BF16 = ml_dtypes.bfloat16
FP8 = ml_dtypes.float8_e4m3

_CACHE = {}


# Tunables (overridable for perf sweeps via _build_program(cfg=...)):
DEFAULT_CFG = {
    "split": (N_DVEB, N_DVE8, N_POOL),  # (dveb, dve8, pool); act = rest
    "emit": "tiles_first",              # or "interleave"
    "pt_bufs": 3,
    "sub": SUB,
    "psums_bufs": 4,
    "first_dma_split": 64,
    "tail_pool_move": 2,
    "tail_pool_lane": "actr",
    "tail_sr_dve": True,
    "gsched": (256, 768, 1024, 1024, 1024, 904),
    "filler_at": (None, None),
    "warmup": 40,
    "lane_sched": None,
    "bufscale": 1.0,
}


def _build_program(cfg=None):
    import concourse.bacc as bacc
    import concourse.mybir as mybir
    import concourse.tile as tile

    cfg = {**DEFAULT_CFG, **(cfg or {})}
    n_dveb, n_dve8, n_pool = cfg["split"]
    n_relu = n_dveb + n_dve8 + n_pool
    n_act = BATCH - n_relu
    rows_dveb = list(range(0, n_dveb))
    rows_dve8 = list(range(n_dveb, n_dveb + n_dve8))
    rows_pool = list(range(n_dveb + n_dve8, n_relu))
    rows_act = list(range(n_relu, BATCH))
    sub = cfg["sub"]

    f32 = mybir.dt.float32
    bf16 = mybir.dt.bfloat16
    fp8 = mybir.dt.float8e4
    AL = mybir.AluOpType
    ACT = mybir.ActivationFunctionType
    DR = mybir.MatmulPerfMode.DoubleRow

    nc = bacc.Bacc("TRN2", target_bir_lowering=False, debug=False, num_devices=N_CORES)
    a_t = nc.declare_dram_parameter("a_t", [128, K_CHUNKS, SHARD_PAD], bf16, isOutput=False)
    w_t = nc.declare_dram_parameter("w_t", [128, 2, K_CHUNKS, 128], bf16, isOutput=False)
    qt = nc.declare_dram_parameter("qt", [128, 2, BATCH], f32, isOutput=False)
    qtn = nc.declare_dram_parameter("qtn", [128, 2, BATCH], f32, isOutput=False)
    stair8 = nc.declare_dram_parameter(
        "stair8", [128, BATCH, 2, BATCH], fp8, isOutput=False)
    st_out = nc.declare_dram_parameter("st_out", [BATCH, SHARD_PAD], f32, isOutput=True)

    # per-lane tile pools so buffer counts track each lane's live-tile needs
    tf = cfg["emit"] == "tiles_first"
    bs = cfg["bufscale"]
    tb_bufs = max(2, int((n_dveb + 2) * bs)) if tf else 6
    t8d_bufs = max(2, int((n_dve8 + 2) * bs)) if tf else 6
    t8p_bufs = max(2, int((n_pool + 2) * bs)) if tf else 4
    t8a_bufs = max(2, int((n_act + 2) * bs)) if tf else 4

    with tile.TileContext(nc) as tc:
        with (
            tc.tile_pool(name="const", bufs=1) as const_pool,
            tc.tile_pool(name="pt", bufs=cfg["pt_bufs"]) as pt_pool,
            tc.tile_pool(name="tb", bufs=tb_bufs) as tb_pool,
            tc.tile_pool(name="t8d", bufs=t8d_bufs) as t8d_pool,
            tc.tile_pool(name="t8p", bufs=t8p_bufs) as t8p_pool,
            tc.tile_pool(name="t8a", bufs=t8a_bufs) as t8a_pool,
            tc.tile_pool(name="sr", bufs=4) as sr_pool,
            tc.tile_pool(name="psumt", bufs=2, space="PSUM") as psumt_pool,
            tc.tile_pool(name="psums", bufs=cfg["psums_bufs"], space="PSUM") as psums_pool,
        ):
            # ---- resident constants ----
            # DMA order matters: w + the first A chunk go first on the sync
            # queue (they gate the first matmul); qt/qtn/stair8 ride the ACT
            # engine's DGE queue in parallel.
            a_all = const_pool.tile([128, K_CHUNKS, SHARD_PAD], bf16)
            fds = cfg["first_dma_split"]
            dma_offs = []
            if fds and fds < sub:
                dma_offs += [(0, fds), (fds, sub - fds)]
            else:
                dma_offs += [(0, sub)]
            for i in range(1, N_GROUPS * (GROUP // sub)):
                dma_offs.append((i * sub, sub))
            nc.sync.dma_start(
                out=a_all[:, :, : dma_offs[0][1]], in_=a_t[:, :, : dma_offs[0][1]])
            w_sb = const_pool.tile([128, 2, K_CHUNKS, 128], bf16)
            nc.scalar.dma_start(out=w_sb[:, 0], in_=w_t[:, 0])
            nc.scalar.dma_start(out=w_sb[:, 1], in_=w_t[:, 1])

            qt_sb = const_pool.tile([128, 2, BATCH], f32)
            nc.gpsimd.dma_start(out=qt_sb[:], in_=qt[:])
            qtn_sb = const_pool.tile([128, 2, BATCH], f32)
            nc.gpsimd.dma_start(out=qtn_sb[:], in_=qtn[:])
            stair8_sb = const_pool.tile([128, BATCH, 2, BATCH], fp8)
            nc.scalar.dma_start(out=stair8_sb[:], in_=stair8[:])
            stairb = const_pool.tile([128, 2 * BATCH - 1], bf16)
            nc.gpsimd.memset(stairb[:], 0.0)
            nc.gpsimd.memset(stairb[:, BATCH - 1 : BATCH], 2.0)
            negones = const_pool.tile([128, BATCH], bf16)
            nc.gpsimd.memset(negones[:], 0.0)
            nc.gpsimd.memset(negones[:, :n_relu], -1.0)

            for o, s in dma_offs[1:]:
                nc.sync.dma_start(
                    out=a_all[:, :, o : o + s], in_=a_t[:, :, o : o + s])

            if cfg["warmup"]:
                # dummy matmuls ramp the PE pstate while the first A/W DMAs
                # are in flight, so the real projection starts at full clock
                wu = const_pool.tile([128, 512], bf16)
                nc.gpsimd.memset(wu[:], 0.0)
                wups = psums_pool.tile([64, 512], f32, tag="psum_c")
                for i in range(cfg["warmup"]):
                    nc.tensor.matmul(
                        wups[:, :64], wu[:, :64], wu[:, :64],
                        start=True, stop=True, skip_group_check=True)

            gsizes = list(cfg["gsched"])
            assert sum(gsizes) == SHARD
            gmax = max(max(gsizes), GROUP)
            n_groups = len(gsizes)
            goffs = [sum(gsizes[:i]) for i in range(n_groups)]
            pt_tiles = [None] * n_groups

            def proj_pieces(g):
                """Projection + PSUM->SBUF copy for group g as per-sub
                emission pieces (interleavable into the PE stream)."""
                g0 = goffs[g]
                gsz = gsizes[g]
                ssz = [sub] * (gsz // sub) + ([gsz % sub] if gsz % sub else [])
                if g == 0 and fds and fds < ssz[0]:
                    ssz = [fds, ssz[0] - fds] + ssz[1:]
                pt_sb = pt_pool.tile([128, 2, gmax], bf16, tag="pt")
                pt_tiles[g] = pt_sb

                soff = [sum(ssz[:i]) for i in range(len(ssz))]

                def piece(s):
                    o = soff[s]
                    ptp = psumt_pool.tile([128, 2, sub], f32, tag="ptp")
                    for h in range(2):
                        for k in range(K_CHUNKS):
                            nc.tensor.matmul(
                                ptp[:, h, : ssz[s]],
                                w_sb[:, h, k, :],
                                a_all[:, k, g0 + o : g0 + o + ssz[s]],
                                start=(k == 0),
                                stop=(k == K_CHUNKS - 1),
                            )
                    nc.scalar.copy(
                        pt_sb[:, :, o : o + ssz[s]], ptp[:, :, : ssz[s]])

                return [lambda s=s: piece(s) for s in range(len(ssz))]

            def emit_rows(g, filler):
                g0 = goffs[g]
                gsz = gsizes[g]
                csz = [sub] * (gsz // sub) + ([gsz % sub] if gsz % sub else [])
                coff = [sum(csz[:i]) for i in range(len(csz))]
                nch = len(csz)
                pt_sb = pt_tiles[g]
                last = g == n_groups - 1

                # per-group lane map. All relu rows (0..n_relu-1) may move
                # freely between the dveb/dve8/pool lanes per group (same
                # stair value + negones + host handling); lane_sched lets the
                # schedule be pool-heavy early and DVE-heavy late so the tail
                # drains at DVE speed.
                ls = cfg["lane_sched"]
                if ls is not None:
                    gb, g8, gp = ls[g]
                    assert gb + g8 + gp == n_relu
                else:
                    gb, g8, gp = n_dveb, n_dve8, n_pool
                    if last and cfg["tail_pool_move"]:
                        m = cfg["tail_pool_move"]
                        gp -= m
                        if cfg["tail_pool_lane"] == "dveb":
                            gb += m
                        elif cfg["tail_pool_lane"] == "dve8":
                            g8 += m
                        else:
                            gp += 0  # rows become "actr" below
                kinds = {}
                for b in range(n_relu):
                    kinds[b] = ("dveb" if b < gb
                                else "dve8" if b < gb + g8 else "pool")
                if ls is None and last and cfg["tail_pool_move"] and \
                        cfg["tail_pool_lane"] == "actr":
                    for b in range(n_relu - cfg["tail_pool_move"], n_relu):
                        kinds[b] = "actr"
                for b in rows_act:
                    kinds[b] = "act"

                def make_tile(b):
                    kind = kinds[b]
                    if kind == "dveb":
                        tl = tb_pool.tile([128, 2, gmax], bf16, tag="tb")
                        eng = nc.vector
                    elif kind == "dve8":
                        tl = t8d_pool.tile([128, 2, gmax], fp8, tag="t8d")
                        eng = nc.vector
                    elif kind == "pool":
                        tl = t8p_pool.tile([128, 2, gmax], fp8, tag="t8p")
                        eng = nc.gpsimd
                    else:
                        tl = t8a_pool.tile([128, 2, gmax], fp8, tag="t8a")
                        eng = None
                    func = ACT.Relu if kinds[b] == "actr" else ACT.Abs
                    for h in range(2):
                        if eng is None:
                            nc.scalar.activation(
                                tl[:, h, :gsz], pt_sb[:, h, :gsz], func,
                                bias=qtn_sb[:, h, b : b + 1], scale=1.0)
                        else:
                            eng.tensor_scalar(
                                out=tl[:, h, :gsz], in0=pt_sb[:, h, :gsz],
                                scalar1=qt_sb[:, h, b : b + 1], scalar2=0.0,
                                op0=AL.subtract, op1=AL.max)
                    return tl

                # production emission: round-robin across lanes
                prod_order = []
                by_kind = {"dveb": [], "dve8": [], "pool": [], "act": [],
                           "actr": []}
                for b in range(BATCH):
                    by_kind[kinds[b]].append(b)
                mx = max(len(v) for v in by_kind.values())
                for i in range(mx):
                    for kind in ("act", "actr", "pool", "dveb", "dve8"):
                        if i < len(by_kind[kind]):
                            prod_order.append(by_kind[kind][i])

                tiles = {}
                for b in prod_order:
                    tiles[b] = make_tile(b)

                # reduction order: estimated tile completion per lane stream
                per_row = {"dveb": 0.66, "dve8": 1.19, "act": 2.08,
                           "actr": 2.08, "pool": 3.04}
                clock = {k: 0.0 for k in per_row}
                done_at = {}
                for b in prod_order:
                    clock[kinds[b]] += per_row[kinds[b]]
                    done_at[b] = clock[kinds[b]]
                red_order = sorted(done_at, key=done_at.get)

                for c in range(nch):
                    lo, sz = coff[c], csz[c]
                    psum_c = psums_pool.tile([BATCH, sub], f32, tag="psum_c")
                    first = True
                    fpos = cfg["filler_at"]
                    filler_at = (fpos[c] if c < len(fpos) else fpos[-1])
                    if filler_at is None:
                        filler_at = max(
                            (i for i, b in enumerate(red_order)
                             if kinds[b] in ("dveb", "dve8")), default=0)
                    for i, b in enumerate(red_order):
                        kind = kinds[b]
                        tl = tiles[b]
                        if kind == "dveb":
                            for h in range(2):
                                nc.tensor.matmul(
                                    psum_c[:, :sz],
                                    stairb[:, BATCH - 1 - b : 2 * BATCH - 1 - b],
                                    tl[:, h, lo : lo + sz],
                                    start=first, stop=False,
                                    skip_group_check=True)
                                first = False
                        else:
                            nc.tensor.matmul(
                                psum_c[:, :sz],
                                stair8_sb[:, b],
                                tl[:, :, lo : lo + sz],
                                start=first, stop=False,
                                perf_mode=DR, skip_group_check=True)
                            first = False
                        if i == filler_at and filler:
                            filler.pop(0)()
                    for h in range(2):
                        nc.tensor.matmul(
                            psum_c[:, :sz],
                            negones[:],
                            pt_sb[:, h, lo : lo + sz],
                            start=False, stop=(h == 1),
                            skip_group_check=True)
                    sr = sr_pool.tile([BATCH, sub], f32, tag="sr")
                    if last and cfg["tail_sr_dve"]:
                        nc.vector.tensor_copy(out=sr[:, :sz], in_=psum_c[:, :sz])
                    else:
                        nc.scalar.copy(sr[:, :sz], psum_c[:, :sz])
                    nc.sync.dma_start(
                        out=st_out[:, g0 + lo : g0 + lo + sz], in_=sr[:, :sz])
                while filler:
                    filler.pop(0)()

            # software pipelining: group g+1's projection pieces are slotted
            # into group g's reduction stream as PE filler work.
            for p in proj_pieces(0):
                p()
            for g in range(n_groups):
                filler = proj_pieces(g + 1) if g + 1 < n_groups else []
                emit_rows(g, filler)

    nc.compile()
    return nc


def _get_program():
    if "nc" not in _CACHE:
        _CACHE["nc"] = _build_program()
    return _CACHE["nc"]


def _host_query_sum(ent_pkl, other_emb, proj_W, batch_input_ids, mp):
    """Exact replica of the reference's query path, on host (64 rows only)."""
    ids = np.concatenate([batch_input_ids[:, :mp], batch_input_ids[:, mp + 1 : 3]], axis=1)
    ids = ids.astype(np.int64)  # [B, 2]
    q = np.empty((BATCH, 2, EMBED_DIM), dtype=np.float32)
    for b in range(BATCH):
        for j in range(2):
            idx = int(ids[b, j])
            if idx == 0:
                row = other_emb[0]
            elif idx <= NUM_ENT:
                row = ent_pkl[idx - 1].astype(np.float32) @ proj_W.T.astype(np.float32)
            else:
                row = other_emb[idx - NUM_ENT]
            q[b, j] = row
    norm = np.sqrt((q * q).sum(-1, keepdims=True))
    q = q / np.maximum(norm, EPS)
    return q.sum(axis=1)  # [B, 256] float32


def kernel(ent_pkl, other_emb, proj_W, batch_input_ids, batch_mask_position, _timing=None):
    from concourse.bass_utils import run_bass_kernel_spmd

    ent_pkl = np.asarray(ent_pkl, dtype=np.float32)
    other_emb = np.asarray(other_emb, dtype=np.float32)
    proj_W = np.asarray(proj_W, dtype=np.float32)
    batch_input_ids = np.asarray(batch_input_ids)
    mp = int(np.asarray(batch_mask_position))

    q_sum = _host_query_sum(ent_pkl, other_emb, proj_W, batch_input_ids, mp)

    # score column 0: entity row = other_emb[0]
    col0 = -np.abs(q_sum - other_emb[0][None, :]).sum(-1)  # [B]

    # ---- device input prep ----
    # w_t[kp, h, k, m] = proj_W.T[128k+kp, 128h+m]
    w_full = np.ascontiguousarray(proj_W.T)  # [768, 256]
    w_np = np.ascontiguousarray(
        w_full.reshape(K_CHUNKS, 128, 2, 128).transpose(1, 2, 0, 3)).astype(BF16)

    # qt[kp, h, b] = q_sum[b, 128h+kp]
    qth = np.transpose(q_sum.T.reshape(2, 128, BATCH), (1, 0, 2))  # [128, 2, 32]
    qt_np = np.ascontiguousarray(qth.astype(np.float32))
    qtn_np = np.ascontiguousarray((-qth).astype(np.float32))

    # per-row DoubleRow stair: 2.0 for relu-identity rows, 1.0 for abs rows
    stair8_np = np.zeros((128, BATCH, 2, BATCH), dtype=FP8)
    for b in range(BATCH):
        stair8_np[:, b, :, b] = FP8(2.0) if b < N_RELU else FP8(1.0)

    a_t_full = ent_pkl.T.astype(BF16)  # [768, 40000]
    in_maps = []
    for c in range(N_CORES):
        shard_t = a_t_full[:, c * SHARD : (c + 1) * SHARD]  # [768, 5000]
        a_np = np.zeros((128, K_CHUNKS, SHARD_PAD), dtype=BF16)
        a_np[:, :, :SHARD] = shard_t.reshape(K_CHUNKS, 128, SHARD).transpose(1, 0, 2)
        in_maps.append({
            "a_t": a_np,
            "w_t": w_np,
            "qt": qt_np,
            "qtn": qtn_np,
            "stair8": stair8_np,
        })

    nc = _get_program()
    kwargs = dict(_timing) if _timing else {}
    res = run_bass_kernel_spmd(nc, in_maps, list(range(N_CORES)), **kwargs)
    if _timing is not None:
        _CACHE["last_results"] = res

    qsum = q_sum.sum(-1).astype(np.float32)  # [B]
    s_ent = np.empty((BATCH, NUM_ENT), dtype=np.float32)
    for c in range(N_CORES):
        sl = slice(c * SHARD, (c + 1) * SHARD)
        s_ent[:, sl] = res.results[c]["st_out"][:, :SHARD]
    # relu-identity rows still need the +sum(q) term
    s_ent[:N_RELU] += qsum[:N_RELU, None]
    out = np.empty((BATCH, NUM_ENT + 1), dtype=np.float32)
    out[:, 0] = col0
    out[:, 1:] = -s_ent
    return out


# revision 42
# speedup vs baseline: 1.1365x; 1.0125x over previous
"""Trainium2 Bass kernel for nn_BLP_52467320487972 (retrieval_knn, L1 scores).

score[b, e] = -sum_d |query_sum[b, d] - E_embed[e, d]|,
E_embed = [other_emb[0]; ent_pkl @ proj_W.T]

Strategy (8 NeuronCores, entity-sharded, 5000(+pad) entities/core):
  host:   exact query_sum [32, 256] (tiny gather + normalize); score column 0;
          per-core ent shard transposed to [768, 5120] bf16.
  device: bf16 projection on the PE produces P.T half-tiles [128d, E];
          the 32 query rows are split across engines to balance load:
            - DVE bf16 lane: relu(P - q) via 4x tensor_scalar; PE column-sums
              each bf16 half-tile with a 2.0-staircase (|x| = 2 relu(x) - x);
            - DVE fp8 lane: same relu emitted fp8e4 (2x mode); ONE fp8
              DoubleRow matmul per row reduces both halves (pair dim) at
              0.5 cycles/column - 4x cheaper than the bf16 reduction;
            - ACT lane: |P - q| directly via activation(Abs, bias=-q) in fp8,
              DoubleRow-reduced (no relu identity, so no colsum correction);
            - Pool (GPSIMD) lane: relu tensor_scalar in fp8, DoubleRow reduce.
          A negones matmul per half adds the "- sum x" correction only to
          relu-lane rows (host later adds their sum(q) term).
          Pipelining: projection + PSUM->SBUF copy run at 512-entity sub
          granularity and group g+1's projection pieces are slotted into
          group g's reduction stream as PE filler; scores drain per 512-entity
          PSUM chunk; a dummy-matmul warmup ramps the PE pstate while the
          first DMAs are in flight; group sizes are small at both ends to
          shorten pipeline fill/drain.
  host:   stitch score columns, negate, prepend column 0.
"""

import sys

for _p in ("/opt/trn_rl_repo", "/root/.axon_site/_ro/trn_rl_repo"):
    if _p not in sys.path:
        sys.path.append(_p)

import numpy as np
import ml_dtypes

NUM_ENT = 40000
NUM_REL = 100
EMBED_DIM = 256
FEAT_DIM = 768
BATCH = 32
N_CORES = 8
SHARD = NUM_ENT // N_CORES          # 5000
SHARD_PAD = 5120                    # 40 tiles of 128
K_CHUNKS = FEAT_DIM // 128          # 6
GROUP = 1024
N_GROUPS = SHARD_PAD // GROUP       # 5
SUB = 512                           # proj/copy/matmul chunk
EPS = 1e-12

# Query-row assignment (relu rows first so the negones columns are a range):
N_DVEB = 8                          # DVE bf16 relu rows (bf16 stair reduce)
N_DVE8 = 11                         # DVE fp8 relu rows (DoubleRow reduce)
N_POOL = 6                          # Pool fp8 relu rows (DoubleRow reduce)
N_ACT = BATCH - N_DVEB - N_DVE8 - N_POOL  # 7 ACT abs rows (DoubleRow reduce)
N_RELU = N_DVEB + N_DVE8 + N_POOL

ROWS_DVEB = list(range(0, N_DVEB))
ROWS_DVE8 = list(range(N_DVEB, N_DVEB + N_DVE8))
ROWS_POOL = list(range(N_DVEB + N_DVE8, N_RELU))
ROWS_ACT = list(range(N_RELU, BATCH))

BF16 = ml_dtypes.bfloat16
FP8 = ml_dtypes.float8_e4m3

_CACHE = {}


# Tunables (overridable for perf sweeps via _build_program(cfg=...)):
DEFAULT_CFG = {
    "split": (N_DVEB, N_DVE8, N_POOL),  # (dveb, dve8, pool); act = rest
    "emit": "tiles_first",              # or "interleave"
    "pt_bufs": 3,
    "sub": SUB,
    "psums_bufs": 4,
    "first_dma_split": 64,
    "tail_pool_move": 0,
    "tail_pool_lane": "actr",
    "tail_sr_dve": True,
    "gsched": (256, 768, 1024, 1024, 1024, 904),
    "filler_at": (None, None),
    "warmup": 40,
    "lane_sched": ((8, 11, 6),) * 5 + ((8, 10, 6, 1),),
    "negones_pos": 20,
    "dma_plan": "cur",
    "proj_fp8": False,
    "bufscale": 1.0,
}


def _build_program(cfg=None):
    import concourse.bacc as bacc
    import concourse.mybir as mybir
    import concourse.tile as tile

    cfg = {**DEFAULT_CFG, **(cfg or {})}
    n_dveb, n_dve8, n_pool = cfg["split"]
    n_relu = n_dveb + n_dve8 + n_pool
    n_act = BATCH - n_relu
    rows_dveb = list(range(0, n_dveb))
    rows_dve8 = list(range(n_dveb, n_dveb + n_dve8))
    rows_pool = list(range(n_dveb + n_dve8, n_relu))
    rows_act = list(range(n_relu, BATCH))
    sub = cfg["sub"]

    f32 = mybir.dt.float32
    bf16 = mybir.dt.bfloat16
    fp8 = mybir.dt.float8e4
    AL = mybir.AluOpType
    ACT = mybir.ActivationFunctionType
    DR = mybir.MatmulPerfMode.DoubleRow

    nc = bacc.Bacc("TRN2", target_bir_lowering=False, debug=False, num_devices=N_CORES)
    pf8 = cfg["proj_fp8"]
    if pf8:
        a_t = nc.declare_dram_parameter(
            "a8", [128, K_CHUNKS // 2, 2, SHARD_PAD], fp8, isOutput=False)
        r_t = nc.declare_dram_parameter(
            "r8", [128, K_CHUNKS // 2, 2, SHARD_PAD], fp8, isOutput=False)
        w_t = nc.declare_dram_parameter(
            "w8", [128, 2, K_CHUNKS // 2, 2, 128], fp8, isOutput=False)
        s_t = nc.declare_dram_parameter(
            "s8", [128, 2, K_CHUNKS // 2, 2, 128], fp8, isOutput=False)
    else:
        a_t = nc.declare_dram_parameter(
            "a_t", [128, K_CHUNKS, SHARD_PAD], bf16, isOutput=False)
        w_t = nc.declare_dram_parameter(
            "w_t", [128, 2, K_CHUNKS, 128], bf16, isOutput=False)
    q2 = nc.declare_dram_parameter("q2", [128, 2, 2, BATCH], f32, isOutput=False)
    stair8 = nc.declare_dram_parameter(
        "stair8", [128, BATCH, 2, BATCH], fp8, isOutput=False)
    st_out = nc.declare_dram_parameter("st_out", [BATCH, SHARD_PAD], f32, isOutput=True)

    # per-lane tile pools so buffer counts track each lane's live-tile needs
    tf = cfg["emit"] == "tiles_first"
    bs = cfg["bufscale"]
    tb_bufs = max(2, int((n_dveb + 2) * bs)) if tf else 6
    t8d_bufs = max(2, int((n_dve8 + 2) * bs)) if tf else 6
    t8p_bufs = max(2, int((n_pool + 2) * bs)) if tf else 4
    t8a_bufs = max(2, int((n_act + 2) * bs)) if tf else 4

    with tile.TileContext(nc) as tc:
        with (
            tc.tile_pool(name="const", bufs=1) as const_pool,
            tc.tile_pool(name="pt", bufs=cfg["pt_bufs"]) as pt_pool,
            tc.tile_pool(name="tb", bufs=tb_bufs) as tb_pool,
            tc.tile_pool(name="t8d", bufs=t8d_bufs) as t8d_pool,
            tc.tile_pool(name="t8p", bufs=t8p_bufs) as t8p_pool,
            tc.tile_pool(name="t8a", bufs=t8a_bufs) as t8a_pool,
            tc.tile_pool(name="sr", bufs=4) as sr_pool,
            tc.tile_pool(name="psumt", bufs=2, space="PSUM") as psumt_pool,
            tc.tile_pool(name="psums", bufs=cfg["psums_bufs"], space="PSUM") as psums_pool,
        ):
            # ---- resident constants ----
            # DMA order matters: w + the first A chunk go first on the sync
            # queue (they gate the first matmul); qt/qtn/stair8 ride the ACT
            # engine's DGE queue in parallel.
            if pf8:
                a_all = const_pool.tile([128, K_CHUNKS // 2, 2, SHARD_PAD], fp8)
                r_all = const_pool.tile([128, K_CHUNKS // 2, 2, SHARD_PAD], fp8)
            else:
                a_all = const_pool.tile([128, K_CHUNKS, SHARD_PAD], bf16)
            fds = cfg["first_dma_split"]
            dma_offs = []
            if fds and fds < sub:
                dma_offs += [(0, fds), (fds, sub - fds)]
            else:
                dma_offs += [(0, sub)]
            for i in range(1, N_GROUPS * (GROUP // sub)):
                dma_offs.append((i * sub, sub))
            if pf8:
                w_sb = const_pool.tile([128, 2, K_CHUNKS // 2, 2, 128], fp8)
                s_sb = const_pool.tile([128, 2, K_CHUNKS // 2, 2, 128], fp8)
                nc.scalar.dma_start(out=s_sb[:], in_=s_t[:])
            else:
                w_sb = const_pool.tile([128, 2, K_CHUNKS, 128], bf16)
            plan = cfg["dma_plan"]
            if plan == "wsync":
                nc.sync.dma_start(out=w_sb[:, 0], in_=w_t[:, 0])
                nc.sync.dma_start(
                    out=a_all[:, :, : dma_offs[0][1]],
                    in_=a_t[:, :, : dma_offs[0][1]])
                nc.scalar.dma_start(out=w_sb[:, 1], in_=w_t[:, 1])
            elif plan == "wsync2":
                nc.sync.dma_start(out=w_sb[:], in_=w_t[:])
                nc.sync.dma_start(
                    out=a_all[:, :, : dma_offs[0][1]],
                    in_=a_t[:, :, : dma_offs[0][1]])
            else:
                if pf8:
                    nc.sync.dma_start(
                        out=a_all[:, :, :, : dma_offs[0][1]],
                        in_=a_t[:, :, :, : dma_offs[0][1]])
                    nc.sync.dma_start(
                        out=r_all[:, :, :, : dma_offs[0][1]],
                        in_=r_t[:, :, :, : dma_offs[0][1]])
                else:
                    nc.sync.dma_start(
                        out=a_all[:, :, : dma_offs[0][1]],
                        in_=a_t[:, :, : dma_offs[0][1]])
                nc.scalar.dma_start(out=w_sb[:, 0], in_=w_t[:, 0])
                nc.scalar.dma_start(out=w_sb[:, 1], in_=w_t[:, 1])

            q2_sb = const_pool.tile([128, 2, 2, BATCH], f32)
            nc.gpsimd.dma_start(out=q2_sb[:], in_=q2[:])
            qt_sb = q2_sb[:, 0]
            qtn_sb = q2_sb[:, 1]
            stair8_sb = const_pool.tile([128, BATCH, 2, BATCH], fp8)
            nc.scalar.dma_start(out=stair8_sb[:], in_=stair8[:])
            stairb = const_pool.tile([128, 2 * BATCH - 1], bf16)
            nc.gpsimd.memset(stairb[:], 0.0)
            nc.gpsimd.memset(stairb[:, BATCH - 1 : BATCH], 2.0)
            negones = const_pool.tile([128, BATCH], bf16)
            nc.gpsimd.memset(negones[:], 0.0)
            nc.gpsimd.memset(negones[:, :n_relu], -1.0)

            for o, s in dma_offs[1:]:
                if pf8:
                    nc.sync.dma_start(
                        out=a_all[:, :, :, o : o + s], in_=a_t[:, :, :, o : o + s])
                    nc.sync.dma_start(
                        out=r_all[:, :, :, o : o + s], in_=r_t[:, :, :, o : o + s])
                else:
                    nc.sync.dma_start(
                        out=a_all[:, :, o : o + s], in_=a_t[:, :, o : o + s])

            if cfg["warmup"]:
                # dummy matmuls ramp the PE pstate while the first A/W DMAs
                # are in flight, so the real projection starts at full clock
                wu = const_pool.tile([128, 512], bf16)
                nc.gpsimd.memset(wu[:], 0.0)
                wups = psums_pool.tile([64, 512], f32, tag="psum_c")
                for i in range(cfg["warmup"]):
                    nc.tensor.matmul(
                        wups[:, :64], wu[:, :64], wu[:, :64],
                        start=True, stop=True, skip_group_check=True)

            gsizes = list(cfg["gsched"])
            assert sum(gsizes) == SHARD
            gmax = max(max(gsizes), GROUP)
            n_groups = len(gsizes)
            goffs = [sum(gsizes[:i]) for i in range(n_groups)]
            pt_tiles = [None] * n_groups

            def proj_pieces(g):
                """Projection + PSUM->SBUF copy for group g as per-sub
                emission pieces (interleavable into the PE stream)."""
                g0 = goffs[g]
                gsz = gsizes[g]
                ssz = [sub] * (gsz // sub) + ([gsz % sub] if gsz % sub else [])
                if g == 0 and fds and fds < ssz[0]:
                    ssz = [fds, ssz[0] - fds] + ssz[1:]
                pt_sb = pt_pool.tile([128, 2, gmax], bf16, tag="pt")
                pt_tiles[g] = pt_sb

                soff = [sum(ssz[:i]) for i in range(len(ssz))]

                def piece(s):
                    o = soff[s]
                    ptp = psumt_pool.tile([128, 2, sub], f32, tag="ptp")
                    if pf8:
                        # 32*P = A8@W8 + R8@W8 + A8@S8 (fp8 DoubleRow,
                        # residual-compensated); un-scaled in the copy.
                        terms = [(w_sb, a_all), (w_sb, r_all), (s_sb, a_all)]
                        for h in range(2):
                            n = len(terms) * (K_CHUNKS // 2)
                            i = 0
                            for wt, at in terms:
                                for j in range(K_CHUNKS // 2):
                                    nc.tensor.matmul(
                                        ptp[:, h, : ssz[s]],
                                        wt[:, h, j],
                                        at[:, j, :, g0 + o : g0 + o + ssz[s]],
                                        start=(i == 0),
                                        stop=(i == n - 1),
                                        perf_mode=DR,
                                    )
                                    i += 1
                        nc.scalar.activation(
                            pt_sb[:, :, o : o + ssz[s]], ptp[:, :, : ssz[s]],
                            ACT.Copy, bias=0.0, scale=1.0 / 32.0)
                    else:
                        for h in range(2):
                            for k in range(K_CHUNKS):
                                nc.tensor.matmul(
                                    ptp[:, h, : ssz[s]],
                                    w_sb[:, h, k, :],
                                    a_all[:, k, g0 + o : g0 + o + ssz[s]],
                                    start=(k == 0),
                                    stop=(k == K_CHUNKS - 1),
                                )
                        nc.scalar.copy(
                            pt_sb[:, :, o : o + ssz[s]], ptp[:, :, : ssz[s]])

                return [lambda s=s: piece(s) for s in range(len(ssz))]

            def emit_rows(g, filler):
                g0 = goffs[g]
                gsz = gsizes[g]
                csz = [sub] * (gsz // sub) + ([gsz % sub] if gsz % sub else [])
                coff = [sum(csz[:i]) for i in range(len(csz))]
                nch = len(csz)
                pt_sb = pt_tiles[g]
                last = g == n_groups - 1

                # per-group lane map. All relu rows (0..n_relu-1) may move
                # freely between the dveb/dve8/pool lanes per group (same
                # stair value + negones + host handling); lane_sched lets the
                # schedule be pool-heavy early and DVE-heavy late so the tail
                # drains at DVE speed.
                ls = cfg["lane_sched"]
                ga = 0
                if ls is not None:
                    if len(ls[g]) == 4:
                        gb, g8, gp, ga = ls[g]
                    else:
                        gb, g8, gp = ls[g]
                    assert gb + g8 + gp + ga == n_relu
                else:
                    gb, g8, gp = n_dveb, n_dve8, n_pool
                    if last and cfg["tail_pool_move"]:
                        m = cfg["tail_pool_move"]
                        gp -= m
                        if cfg["tail_pool_lane"] == "dveb":
                            gb += m
                        elif cfg["tail_pool_lane"] == "dve8":
                            g8 += m
                        else:
                            gp += 0  # rows become "actr" below
                kinds = {}
                for b in range(n_relu):
                    kinds[b] = ("dveb" if b < gb
                                else "dve8" if b < gb + g8 else "pool")
                if ga:
                    for b in range(n_relu - ga, n_relu):
                        kinds[b] = "actr"
                if ls is None and last and cfg["tail_pool_move"] and \
                        cfg["tail_pool_lane"] == "actr":
                    for b in range(n_relu - cfg["tail_pool_move"], n_relu):
                        kinds[b] = "actr"
                for b in rows_act:
                    kinds[b] = "act"

                def make_tile(b):
                    kind = kinds[b]
                    if kind == "dveb":
                        tl = tb_pool.tile([128, 2, gmax], bf16, tag="tb")
                        eng = nc.vector
                    elif kind == "dve8":
                        tl = t8d_pool.tile([128, 2, gmax], fp8, tag="t8d")
                        eng = nc.vector
                    elif kind == "pool":
                        tl = t8p_pool.tile([128, 2, gmax], fp8, tag="t8p")
                        eng = nc.gpsimd
                    else:
                        tl = t8a_pool.tile([128, 2, gmax], fp8, tag="t8a")
                        eng = None
                    func = ACT.Relu if kinds[b] == "actr" else ACT.Abs
                    for h in range(2):
                        if eng is None:
                            nc.scalar.activation(
                                tl[:, h, :gsz], pt_sb[:, h, :gsz], func,
                                bias=qtn_sb[:, h, b : b + 1], scale=1.0)
                        else:
                            eng.tensor_scalar(
                                out=tl[:, h, :gsz], in0=pt_sb[:, h, :gsz],
                                scalar1=qt_sb[:, h, b : b + 1], scalar2=0.0,
                                op0=AL.subtract, op1=AL.max)
                    return tl

                # production emission: round-robin across lanes
                prod_order = []
                by_kind = {"dveb": [], "dve8": [], "pool": [], "act": [],
                           "actr": []}
                for b in range(BATCH):
                    by_kind[kinds[b]].append(b)
                mx = max(len(v) for v in by_kind.values())
                for i in range(mx):
                    for kind in ("act", "actr", "pool", "dveb", "dve8"):
                        if i < len(by_kind[kind]):
                            prod_order.append(by_kind[kind][i])

                tiles = {}
                for b in prod_order:
                    tiles[b] = make_tile(b)

                # reduction order: estimated tile completion per lane stream
                per_row = {"dveb": 0.66, "dve8": 1.19, "act": 2.08,
                           "actr": 2.08, "pool": 3.04}
                clock = {k: 0.0 for k in per_row}
                done_at = {}
                for b in prod_order:
                    clock[kinds[b]] += per_row[kinds[b]]
                    done_at[b] = clock[kinds[b]]
                red_order = sorted(done_at, key=done_at.get)

                for c in range(nch):
                    lo, sz = coff[c], csz[c]
                    psum_c = psums_pool.tile([BATCH, sub], f32, tag="psum_c")

                    # build the chunk's PE emitter list: reductions in
                    # completion order, negones correction at negones_pos
                    # (filler for the tile-starved stretch) or at the end.
                    emitters = []
                    for b in red_order:
                        kind = kinds[b]
                        tl = tiles[b]
                        if kind == "dveb":
                            for h in range(2):
                                emitters.append(("red_bf", b, tl, h))
                        else:
                            emitters.append(("red_dr", b, tl, None))
                    npos = cfg["negones_pos"]
                    if npos is None:
                        npos = len(emitters)
                    npos = min(npos, len(emitters))
                    emitters = (emitters[:npos]
                                + [("neg", None, None, h) for h in range(2)]
                                + emitters[npos:])

                    fpos = cfg["filler_at"]
                    filler_at = (fpos[c] if c < len(fpos) else fpos[-1])
                    if filler_at is None:
                        filler_at = max(
                            (i for i, e in enumerate(emitters)
                             if e[0] in ("red_bf", "red_dr")
                             and kinds.get(e[1], "") in ("dveb", "dve8")),
                            default=0)
                    for i, (op, b, tl, h) in enumerate(emitters):
                        first = i == 0
                        stop = i == len(emitters) - 1
                        if op == "red_bf":
                            nc.tensor.matmul(
                                psum_c[:, :sz],
                                stairb[:, BATCH - 1 - b : 2 * BATCH - 1 - b],
                                tl[:, h, lo : lo + sz],
                                start=first, stop=stop,
                                skip_group_check=True)
                        elif op == "red_dr":
                            nc.tensor.matmul(
                                psum_c[:, :sz],
                                stair8_sb[:, b],
                                tl[:, :, lo : lo + sz],
                                start=first, stop=stop,
                                perf_mode=DR, skip_group_check=True)
                        else:  # negones correction pass
                            nc.tensor.matmul(
                                psum_c[:, :sz],
                                negones[:],
                                pt_sb[:, h, lo : lo + sz],
                                start=first, stop=stop,
                                skip_group_check=True)
                        if i == filler_at and filler:
                            filler.pop(0)()
                    sr = sr_pool.tile([BATCH, sub], f32, tag="sr")
                    if last and cfg["tail_sr_dve"]:
                        nc.vector.tensor_copy(out=sr[:, :sz], in_=psum_c[:, :sz])
                    else:
                        nc.scalar.copy(sr[:, :sz], psum_c[:, :sz])
                    nc.sync.dma_start(
                        out=st_out[:, g0 + lo : g0 + lo + sz], in_=sr[:, :sz])
                while filler:
                    filler.pop(0)()

            # software pipelining: group g+1's projection pieces are slotted
            # into group g's reduction stream as PE filler work.
            for p in proj_pieces(0):
                p()
            for g in range(n_groups):
                filler = proj_pieces(g + 1) if g + 1 < n_groups else []
                emit_rows(g, filler)

    nc.compile()
    return nc


def _get_program():
    if "nc" not in _CACHE:
        _CACHE["nc"] = _build_program()
    return _CACHE["nc"]


def _host_query_sum(ent_pkl, other_emb, proj_W, batch_input_ids, mp):
    """Exact replica of the reference's query path, on host (64 rows only)."""
    ids = np.concatenate([batch_input_ids[:, :mp], batch_input_ids[:, mp + 1 : 3]], axis=1)
    ids = ids.astype(np.int64)  # [B, 2]
    q = np.empty((BATCH, 2, EMBED_DIM), dtype=np.float32)
    for b in range(BATCH):
        for j in range(2):
            idx = int(ids[b, j])
            if idx == 0:
                row = other_emb[0]
            elif idx <= NUM_ENT:
                row = ent_pkl[idx - 1].astype(np.float32) @ proj_W.T.astype(np.float32)
            else:
                row = other_emb[idx - NUM_ENT]
            q[b, j] = row
    norm = np.sqrt((q * q).sum(-1, keepdims=True))
    q = q / np.maximum(norm, EPS)
    return q.sum(axis=1)  # [B, 256] float32


def kernel(ent_pkl, other_emb, proj_W, batch_input_ids, batch_mask_position, _timing=None):
    from concourse.bass_utils import run_bass_kernel_spmd

    ent_pkl = np.asarray(ent_pkl, dtype=np.float32)
    other_emb = np.asarray(other_emb, dtype=np.float32)
    proj_W = np.asarray(proj_W, dtype=np.float32)
    batch_input_ids = np.asarray(batch_input_ids)
    mp = int(np.asarray(batch_mask_position))

    q_sum = _host_query_sum(ent_pkl, other_emb, proj_W, batch_input_ids, mp)

    # score column 0: entity row = other_emb[0]
    col0 = -np.abs(q_sum - other_emb[0][None, :]).sum(-1)  # [B]

    # ---- device input prep ----
    pf8 = DEFAULT_CFG["proj_fp8"]
    w_full = np.ascontiguousarray(proj_W.T)  # [768, 256]
    if pf8:
        ws = (w_full * 32.0).astype(np.float32)
        w8_f = ws.astype(FP8)
        s8_f = (ws - w8_f.astype(np.float32)).astype(FP8)

        def _wlay(w):  # [768, 256] -> [128, 2, 3, 2, 128]
            return np.ascontiguousarray(
                w.reshape(K_CHUNKS // 2, 2, 128, 2, 128).transpose(2, 3, 0, 1, 4))

        w8_np, s8_np = _wlay(w8_f), _wlay(s8_f)
    else:
        w_np = np.ascontiguousarray(
            w_full.reshape(K_CHUNKS, 128, 2, 128).transpose(1, 2, 0, 3)).astype(BF16)

    # q2[kp, sign, h, b] = (+-) q_sum[b, 128h+kp]
    qth = np.transpose(q_sum.T.reshape(2, 128, BATCH), (1, 0, 2))  # [128, 2, 32]
    q2_np = np.ascontiguousarray(
        np.stack([qth, -qth], axis=1).astype(np.float32))  # [128, 2, 2, 32]

    # per-row DoubleRow stair: 2.0 for relu-identity rows, 1.0 for abs rows
    stair8_np = np.zeros((128, BATCH, 2, BATCH), dtype=FP8)
    for b in range(BATCH):
        stair8_np[:, b, :, b] = FP8(2.0) if b < N_RELU else FP8(1.0)

    in_maps = []
    if pf8:
        at_f32 = ent_pkl.T  # [768, 40000] f32
        a8_full = at_f32.astype(FP8)
        r8_full = (at_f32 - a8_full.astype(np.float32)).astype(FP8)
        for c in range(N_CORES):
            m = {"w8": w8_np, "s8": s8_np, "q2": q2_np, "stair8": stair8_np}
            for nm, full in (("a8", a8_full), ("r8", r8_full)):
                shard_t = full[:, c * SHARD : (c + 1) * SHARD]
                arr = np.zeros((128, K_CHUNKS // 2, 2, SHARD_PAD), dtype=FP8)
                arr[:, :, :, :SHARD] = shard_t.reshape(
                    K_CHUNKS // 2, 2, 128, SHARD).transpose(2, 0, 1, 3)
                m[nm] = arr
            in_maps.append(m)
    else:
        a_t_full = ent_pkl.T.astype(BF16)  # [768, 40000]
        for c in range(N_CORES):
            shard_t = a_t_full[:, c * SHARD : (c + 1) * SHARD]  # [768, 5000]
            a_np = np.zeros((128, K_CHUNKS, SHARD_PAD), dtype=BF16)
            a_np[:, :, :SHARD] = shard_t.reshape(
                K_CHUNKS, 128, SHARD).transpose(1, 0, 2)
            in_maps.append({
                "a_t": a_np,
                "w_t": w_np,
                "q2": q2_np,
                "stair8": stair8_np,
            })

    nc = _get_program()
    kwargs = dict(_timing) if _timing else {}
    res = run_bass_kernel_spmd(nc, in_maps, list(range(N_CORES)), **kwargs)
    if _timing is not None:
        _CACHE["last_results"] = res

    qsum = q_sum.sum(-1).astype(np.float32)  # [B]
    s_ent = np.empty((BATCH, NUM_ENT), dtype=np.float32)
    for c in range(N_CORES):
        sl = slice(c * SHARD, (c + 1) * SHARD)
        s_ent[:, sl] = res.results[c]["st_out"][:, :SHARD]
    # relu-identity rows still need the +sum(q) term
    s_ent[:N_RELU] += qsum[:N_RELU, None]
    out = np.empty((BATCH, NUM_ENT + 1), dtype=np.float32)
    out[:, 0] = col0
    out[:, 1:] = -s_ent
    return out
